# revision 1
# baseline (speedup 1.0000x reference)
"""Multi-head cross-attention on 8 Trainium2 NeuronCores.

Sharding: data-parallel over batch (2) x tensor-parallel over heads (4 groups
of 4 heads). Core c handles batch c//4, head-group c%4 (a 256-wide slice of
the QKV projection space). Each core computes a partial output-projection
Y_partial = ctx_c @ Wo_c; a ReduceScatter(add) over each batch's 4 cores
leaves each core with a 512-row shard of the summed output, which the host
concatenates.

On-core dataflow (all matmuls in fp32r at full PE rate):
  - x is PE-transposed to d-major (in two d-halves to halve SBUF residency;
    projections accumulate the halves via an SBUF add). Q^T/K^T = W.T @ x^T
    come out j-major, V = x @ Wv comes out s-major -- exactly the operand
    layouts the attention matmuls need, so no other transposes occur.
  - scores are built k-major (S^T) two PSUM banks at a time, exp'd in one
    [128,1024] scalar-engine op (no max subtraction: scores ~ N(0,1)), and
    fed straight into the PV matmul. V carries 64 ones-columns so the softmax
    denominator lands in PSUM partitions 64..127 of the same matmul; a single
    PSUM-to-PSUM tensor divide normalizes while evicting to SBUF.
  - bq/bk are applied on-device (per-partition bias in j-major layout).
    bv/bo commute through softmax/out-projection exactly (softmax rows sum
    to 1), so the host adds bv @ Wo + bo to the final output.
"""

import numpy as np

B, SEQ, D, H, DH = 2, 2048, 1024, 16, 64
N_CORES = 8
GROUPS = 4            # head-groups per batch (cores per batch)
JG = D // GROUPS      # 256 projection dims per core
HPC = H // GROUPS     # 4 heads per core
P = 128

_cached = {}


def _build_program(seq=SEQ, use_f32r=True, with_collective=True,
                   exp_width=1024):
    import concourse.tile as tile
    from concourse import bacc, mybir
    from concourse.masks import make_identity

    F32 = mybir.dt.float32
    MMT = mybir.dt.float32r if use_f32r else mybir.dt.float32

    def mm(x):
        return x.bitcast(MMT)

    # producers of matmul operands must write rounded f32r (walrus birverifier)
    r = mm

    s_chunks = seq // P          # 16  (128-row chunks)
    sb_chunks = seq // 512       # 4   (512-wide blocks)
    sk_chunks = seq // 1024      # 2   (1024-wide attention blocks)
    d_chunks = D // P            # 8
    dh_chunks = d_chunks // 2    # 4   (per d-half)
    j_chunks = JG // P           # 2

    nc = bacc.Bacc("TRN2", target_bir_lowering=False, debug=False,
                   num_devices=N_CORES)

    x1b = nc.dram_tensor("x1b", [seq, D], F32, kind="ExternalInput")
    x2b = nc.dram_tensor("x2b", [seq, D], F32, kind="ExternalInput")
    wq = nc.dram_tensor("wq", [D, JG], F32, kind="ExternalInput")
    wk = nc.dram_tensor("wk", [D, JG], F32, kind="ExternalInput")
    wv = nc.dram_tensor("wv", [D, JG], F32, kind="ExternalInput")
    wo = nc.dram_tensor("wo", [JG, D], F32, kind="ExternalInput")
    bqr = nc.dram_tensor("bqr", [P, j_chunks], F32, kind="ExternalInput")
    bkr = nc.dram_tensor("bkr", [P, j_chunks], F32, kind="ExternalInput")
    y_out = nc.dram_tensor("y_out", [seq // GROUPS, D], F32,
                           kind="ExternalOutput")

    EXP = mybir.ActivationFunctionType.Exp
    DIV = mybir.AluOpType.divide

    with tile.TileContext(nc) as tc:
        with (
            tc.tile_pool(name="consts", bufs=1) as consts,
            tc.tile_pool(name="wqkv", bufs=3) as wqkv_pool,
            tc.tile_pool(name="wop", bufs=1) as wo_pool,
            tc.tile_pool(name="xload", bufs=3) as xload,
            tc.tile_pool(name="xt", bufs=2) as xt_pool,
            tc.tile_pool(name="acts", bufs=1) as acts,
            tc.tile_pool(name="ctp", bufs=2) as ct_pool,
            tc.tile_pool(name="epool", bufs=4) as epool,
            tc.tile_pool(name="small", bufs=2) as small,
            tc.tile_pool(name="ysb", bufs=4) as ysb,
            tc.tile_pool(name="psum_mm", bufs=2, space="PSUM") as psum_mm,
            tc.tile_pool(name="psum_s", bufs=(2 if exp_width == 1024 else 4), space="PSUM") as psum_s,
            tc.tile_pool(name="psum_u", bufs=2, space="PSUM") as psum_u,
            tc.tile_pool(name="dram", bufs=1, space="DRAM") as dram,
        ):
            ident = consts.tile([P, P], F32)
            make_identity(nc, ident)

            def load_weight_cast(wsb, w_dram, n_outer, width, pat):
                # DMA f32 chunks then cast into the f32r operand tile
                for o in range(n_outer):
                    st = ysb.tile([P, 1024], F32, tag="y",
                                  name=f"wst_{wsb.name}_{o}")
                    nc.sync.dma_start(
                        st[:, :width],
                        w_dram.rearrange(pat, p=P)[:, o, :])
                    nc.vector.tensor_copy(r(wsb[:, o, :]), st[:, :width])

            def load_slab(x_dram, sb):
                # two 1MB DMAs per slab; tile q-pair layout [P, 2, D]
                pairs = []
                for g in range(2):
                    xt_ = xload.tile([P, 2, D], F32, tag="xload")
                    nc.sync.dma_start(
                        xt_[:],
                        x_dram[(sb * 4 + 2 * g) * P:(sb * 4 + 2 * g + 2) * P,
                               :].rearrange("(q p) d -> p q d", p=P))
                    pairs.append(xt_)
                return [pairs[q // 2][:, q % 2] for q in range(4)]

            def transpose_slab(x_dram, sb, use_act=False, xts=None):
                # x rows [sb*512, (sb+1)*512) x full D -> xT [P, d_chunks, 512]
                # (d-major). In phase A (use_act) the idle 2-bank score slots
                # hold 8 batched PE transposes evicted by ONE [128,1024] copy,
                # alternating ACT/DVE; during attention (x1) fall back to
                # single-bank "mm" tiles so the score slots stay free.
                if xts is None:
                    xts = load_slab(x_dram, sb)
                xT = xt_pool.tile([P, d_chunks, 512], F32, tag="xT")
                if use_act:
                    for dg in range(d_chunks // 2):
                        pt = psum_s.tile([P, 1024], F32, tag="s",
                                         name=f"ptx_{x_dram.name}_{sb}_{dg}")
                        for i in range(2):
                            dc = 2 * dg + i
                            for q in range(4):
                                nc.tensor.transpose(
                                    pt[:, i * 512 + q * P:
                                       i * 512 + (q + 1) * P],
                                    xts[q][:, dc * P:(dc + 1) * P], ident[:])
                        out2 = xT[:, 2 * dg:2 * dg + 2, :]
                        if dg % 2 == 1:
                            nc.scalar.copy(r(out2), pt[:])
                        else:
                            nc.vector.tensor_copy(r(out2), pt[:])
                else:
                    for dc in range(d_chunks):
                        pt = psum_mm.tile([P, 512], F32, tag="mm")
                        for q in range(4):
                            nc.tensor.transpose(
                                pt[:, q * P:(q + 1) * P],
                                xts[q][:, dc * P:(dc + 1) * P], ident[:])
                        nc.vector.tensor_copy(r(xT[:, dc, :]), pt[:])
                return xT

            # x2 slab 0 loads go first so transposes start immediately;
            # weight DMAs stream in behind them
            x2tiles0 = load_slab(x2b, 0)

            # qkv weights rotate through 2 shared slots (k, v, then q)
            wk_sb = wqkv_pool.tile([P, d_chunks, JG], F32, tag="wqkv")
            wv_sb = wqkv_pool.tile([P, d_chunks, JG], F32, tag="wqkv")
            wo_sb = wo_pool.tile([P, j_chunks, D], F32, tag="wo")
            load_weight_cast(wk_sb, wk, d_chunks, JG, "(o p) j -> p o j")
            load_weight_cast(wv_sb, wv, d_chunks, JG, "(o p) j -> p o j")
            load_weight_cast(wo_sb, wo, j_chunks, D, "(o p) n -> p o n")
            bq_sb = consts.tile([P, j_chunks], F32, tag="bq")
            bk_sb = consts.tile([P, j_chunks], F32, tag="bk")
            nc.sync.dma_start(bq_sb[:], bqr[:])
            nc.sync.dma_start(bk_sb[:], bkr[:])


            kT = acts.tile([P, j_chunks, seq], F32, tag="kT")
            qT = acts.tile([P, j_chunks, seq], F32, tag="qT")
            # V'' per head-column-block: cols 0..63 V_h, 64..127 ones
            vpp = acts.tile([P, s_chunks, HPC * P], F32, tag="vpp")

            ones_f32 = consts.tile([P, DH], F32, tag="ones")
            nc.vector.memset(ones_f32[:], 1.0)
            for si in range(s_chunks):
                ones_view = vpp[:, si].rearrange("p (h q) -> p h q", q=P)[:, :, DH:P]
                nc.vector.tensor_copy(
                    r(ones_view),
                    ones_f32[:, None, :].to_broadcast([P, HPC, DH]))

            def project_jmajor(xT_s, w_sb, sb, out, bias, use_act=False,
                               on_s=False):
                # out[j, sb-slab] = w.T @ xT_s + bias. on_s borrows the
                # attention score PSUM banks (idle before the first exp) so
                # projections pipeline in parallel with the next slab's
                # transposes instead of contending for the 2 "mm" slots.
                ssl = slice(sb * 512, (sb + 1) * 512)
                for jc in range(j_chunks):
                    if on_s:
                        pk = psum_s.tile([P, 512], F32, tag="s",
                                         name=f"pk_{w_sb.name}_{sb}_{jc}")
                    else:
                        pk = psum_mm.tile([P, 512], F32, tag="mm")
                    for dc in range(d_chunks):
                        nc.tensor.matmul(
                            pk[:],
                            mm(w_sb[:, dc, jc * P:(jc + 1) * P]),
                            mm(xT_s[:, dc, :]),
                            start=(dc == 0), stop=(dc == d_chunks - 1))
                    if use_act:
                        nc.scalar.add(r(out[:, jc, ssl]), pk[:],
                                      bias[:, jc:jc + 1])
                    else:
                        nc.vector.tensor_scalar_add(
                            r(out[:, jc, ssl]), pk[:], bias[:, jc:jc + 1])

            def project_v(xT_s, sb):
                # V[s-slab, j] = x2_slab @ Wv into the vpp head blocks
                for q in range(4):
                    si = sb * 4 + q
                    pv = psum_u.tile([P, JG], F32, tag="u")
                    for dc in range(d_chunks):
                        nc.tensor.matmul(
                            pv[:],
                            mm(xT_s[:, dc, q * P:(q + 1) * P]),
                            mm(wv_sb[:, dc, :]),
                            start=(dc == 0), stop=(dc == d_chunks - 1))
                    vv = vpp[:, si].rearrange("p (h q) -> p h q", q=P)[:, :, 0:DH]
                    nc.vector.tensor_copy(
                        r(vv), pv[:].rearrange("p (h q) -> p h q", q=DH))

            ybounce = dram.tile([seq, D], F32, tag="yin")

            cts = {}
            pus_by = {}

            def emit_oproj(sc, cT):
                for s8 in range(8):
                  with nc.named_scope("oproj"):
                    si = sc * 8 + s8
                    yt = ysb.tile([P, D], F32, tag="y",
                                  name=f"yt_{sc}_{s8}")
                    last = sc == sk_chunks - 1
                    for nck in range(2):
                        if last and (s8 * 2 + nck) % 2 == 1:
                            py = psum_s.tile([P, 512], F32, tag="s",
                                             name=f"py_{sc}_{s8}_{nck}")
                        else:
                            py = psum_mm.tile([P, 512], F32, tag="mm",
                                              name=f"py_{sc}_{s8}_{nck}")
                        for jc in range(j_chunks):
                            nc.tensor.matmul(
                                py[:],
                                mm(cT[:, jc, s8 * P:(s8 + 1) * P]),
                                mm(wo_sb[:, jc, nck * 512:(nck + 1) * 512]),
                                start=(jc == 0), stop=(jc == j_chunks - 1))
                        if last:
                            nc.scalar.copy(
                                yt[:, nck * 512:(nck + 1) * 512], py[:])
                        else:
                            nc.vector.tensor_copy(
                                yt[:, nck * 512:(nck + 1) * 512], py[:])
                    nc.sync.dma_start(ybounce[si * P:(si + 1) * P, :], yt[:])

            def emit_pv(sc, h, kc, et):
                jc, po = h // 2, (h % 2) * DH
                if kc == 0:
                    pus_by[(sc, h)] = [
                        psum_u.tile([P, 512], F32, tag="u",
                                    name=f"pu_{sc}_{h}_{i}")
                        for i in range(2)]
                pus = pus_by[(sc, h)]
                for half in range(2):
                    fsl = slice(half * 512, (half + 1) * 512)
                    nc.tensor.matmul(
                        pus[half][:],
                        mm(vpp[:, kc, h * P:(h + 1) * P]),
                        mm(et[:, fsl]),
                        start=(kc == 0), stop=(kc == s_chunks - 1))
                if kc == s_chunks - 1:
                    cT = cts[sc]
                    for half in range(2):
                        fsl = slice(half * 512, (half + 1) * 512)
                        rt = small.tile([DH, 512], F32, tag="rt",
                                        name=f"rt_{sc}_{h}_{half}")
                        nc.vector.reciprocal(rt[:], pus[half][DH:P, :])
                        nc.vector.tensor_mul(
                            r(cT[po:po + DH, jc, fsl]),
                            pus[half][0:DH, :], rt[:])
                    del pus_by[(sc, h)]
                    if h == HPC - 1:
                        emit_oproj(sc, cT)

            pend = []

            def emit_attn_unit(sc, h, kc):
              with nc.named_scope("attn"):
                if (h, kc) == (0, 0):
                    cts[sc] = ct_pool.tile([P, j_chunks, 1024], F32,
                                           tag="cT", name=f"cT_{sc}")
                jc, po = h // 2, (h % 2) * DH
                ps = psum_s.tile([P, 1024], F32, tag="s",
                                 name=f"ps_{sc}_{h}_{kc}")
                for half in range(2):
                    hsl = slice(sc * 1024 + half * 512,
                                sc * 1024 + (half + 1) * 512)
                    nc.tensor.matmul(
                        ps[:, half * 512:(half + 1) * 512],
                        mm(kT[po:po + DH, jc, kc * P:(kc + 1) * P]),
                        mm(qT[po:po + DH, jc, hsl]),
                        start=True, stop=True)
                et = epool.tile([P, 1024], F32, tag="e",
                                name=f"et_{sc}_{h}_{kc}")
                nc.scalar.activation(r(et[:]), ps[:], EXP, scale=0.125)
                pend.append((sc, h, kc, et))
                if len(pend) > 3:
                    emit_pv(*pend.pop(0))

            # ---- x2 -> K^T, V'' (per 512-row slab) ----
            for sb in range(sb_chunks):
                with nc.named_scope("x2t"):
                    x2T_s = transpose_slab(x2b, sb, use_act=True,
                                           xts=(x2tiles0 if sb == 0 else None))
                with nc.named_scope("kproj"):
                    project_jmajor(x2T_s, wk_sb, sb, kT, bk_sb, use_act=True)
                with nc.named_scope("vproj"):
                    project_v(x2T_s, sb)

            # ---- x1 -> Q^T (per slab; overlaps with attention below) ----
            wq_sb = wqkv_pool.tile([P, d_chunks, JG], F32, tag="wqkv")
            load_weight_cast(wq_sb, wq, d_chunks, JG, "(o p) j -> p o j")
            for sb in range(sb_chunks):
                with nc.named_scope("x1t"):
                    x1T_s = transpose_slab(x1b, sb)
                with nc.named_scope("qproj"):
                    project_jmajor(x1T_s, wq_sb, sb, qT, bq_sb, on_s=(sb < 2))

            # ---- attention units (flat, PV lagging exp by 2) ----
            for sc in range(sk_chunks):
                for h in range(HPC):
                    for kc in range(s_chunks):
                        emit_attn_unit(sc, h, kc)
            with nc.named_scope("attn"):
                for args in pend:
                    emit_pv(*args)

            # ---- sum partials across the 4 cores of this batch ----
            # Two half-sized ReduceScatters: the first depends only on the
            # first 1024 rows (written when attention chunk 0's out-projection
            # lands), so it overlaps chunk 1's attention instead of
            # serializing after all compute.
            if with_collective:
                half = seq // 2                 # 1024 rows per collective
                qr = seq // GROUPS // 2         # 256 rows per rank per half
                for ci in range(2):
                    ysc = dram.tile([qr, D], F32, tag="yout",
                                    name=f"ysc_{ci}")
                    nc.gpsimd.collective_compute(
                        "ReduceScatter",
                        mybir.AluOpType.add,
                        replica_groups=[[0, 1, 2, 3], [4, 5, 6, 7]],
                        ins=[ybounce[ci * half:(ci + 1) * half, :].opt()],
                        outs=[ysc[:].opt()],
                    )
                    nc.sync.dma_start(y_out[ci * qr:(ci + 1) * qr, :], ysc[:])
            else:
                nc.sync.dma_start(y_out[:], ybounce[:seq // GROUPS, :])

    nc.compile()
    return nc


def _get_program(seq=SEQ, use_f32r=True):
    key = (seq, use_f32r)
    if key not in _cached:
        _cached[key] = _build_program(seq, use_f32r)
    return _cached[key]


def make_in_maps(x1, x2, Wq, bq, Wk, bk, Wv, bv, Wo, bo):
    """Per-core input dicts for the SPMD program."""
    in_maps = []
    for c in range(N_CORES):
        b, g = c // GROUPS, c % GROUPS
        js = slice(g * JG, (g + 1) * JG)
        in_maps.append({
            "x1b": np.ascontiguousarray(x1[b]),
            "x2b": np.ascontiguousarray(x2[b]),
            "wq": np.ascontiguousarray(Wq[:, js]),
            "wk": np.ascontiguousarray(Wk[:, js]),
            "wv": np.ascontiguousarray(Wv[:, js]),
            "wo": np.ascontiguousarray(Wo[js, :]),
            "bqr": np.ascontiguousarray(bq[js].reshape(2, P).T),
            "bkr": np.ascontiguousarray(bk[js].reshape(2, P).T),
        })
    return in_maps


def assemble(results, Wv_bias_fix):
    """results: list of per-core {'y_out': [seq//GROUPS, D]}.

    y_out rows [0:q) = rank's quarter of input rows [0:seq/2);
    rows [q:2q) = rank's quarter of input rows [seq/2:seq)."""
    seq = results[0]["y_out"].shape[0] * GROUPS
    q = seq // GROUPS // 2
    Y = np.empty((B, seq, D), np.float32)
    for c in range(N_CORES):
        b, rr = c // GROUPS, c % GROUPS
        yo = results[c]["y_out"]
        Y[b, rr * q:(rr + 1) * q, :] = yo[:q]
        Y[b, seq // 2 + rr * q:seq // 2 + (rr + 1) * q, :] = yo[q:]
    Y += Wv_bias_fix
    return Y


def kernel(x1, x2, Wq, bq, Wk, bk, Wv, bv, Wo, bo):
    from concourse.bass_utils import run_bass_kernel_spmd

    x1 = np.asarray(x1, np.float32)
    x2 = np.asarray(x2, np.float32)
    Wq, bq = np.asarray(Wq, np.float32), np.asarray(bq, np.float32)
    Wk, bk = np.asarray(Wk, np.float32), np.asarray(bk, np.float32)
    Wv, bv = np.asarray(Wv, np.float32), np.asarray(bv, np.float32)
    Wo, bo = np.asarray(Wo, np.float32), np.asarray(bo, np.float32)

    nc = _get_program(SEQ)
    in_maps = make_in_maps(x1, x2, Wq, bq, Wk, bk, Wv, bv, Wo, bo)
    res = run_bass_kernel_spmd(nc, in_maps, core_ids=list(range(N_CORES)))
    fix = (bv @ Wo + bo).astype(np.float32)
    return assemble(res.results, fix)



# revision 8
# speedup vs baseline: 1.0187x; 1.0187x over previous
"""Multi-head cross-attention on 8 Trainium2 NeuronCores.

Sharding: data-parallel over batch (2) x tensor-parallel over heads (4 groups
of 4 heads). Core c handles batch c//4, head-group c%4 (a 256-wide slice of
the QKV projection space). Each core computes a partial output-projection
Y_partial = ctx_c @ Wo_c; a ReduceScatter(add) over each batch's 4 cores
leaves each core with a 512-row shard of the summed output, which the host
concatenates.

On-core dataflow (all matmuls in fp32r at full PE rate):
  - x is PE-transposed to d-major (in two d-halves to halve SBUF residency;
    projections accumulate the halves via an SBUF add). Q^T/K^T = W.T @ x^T
    come out j-major, V = x @ Wv comes out s-major -- exactly the operand
    layouts the attention matmuls need, so no other transposes occur.
  - scores are built k-major (S^T) two PSUM banks at a time, exp'd in one
    [128,1024] scalar-engine op (no max subtraction: scores ~ N(0,1)), and
    fed straight into the PV matmul. V carries 64 ones-columns so the softmax
    denominator lands in PSUM partitions 64..127 of the same matmul; a single
    PSUM-to-PSUM tensor divide normalizes while evicting to SBUF.
  - bq/bk are applied on-device (per-partition bias in j-major layout).
    bv/bo commute through softmax/out-projection exactly (softmax rows sum
    to 1), so the host adds bv @ Wo + bo to the final output.
"""

import numpy as np

B, SEQ, D, H, DH = 2, 2048, 1024, 16, 64
N_CORES = 8
GROUPS = 4            # head-groups per batch (cores per batch)
JG = D // GROUPS      # 256 projection dims per core
HPC = H // GROUPS     # 4 heads per core
P = 128

_cached = {}


def _build_program(seq=SEQ, use_f32r=True, with_collective=True,
                   exp_width=1024):
    import concourse.tile as tile
    from concourse import bacc, mybir
    from concourse.masks import make_identity

    F32 = mybir.dt.float32
    MMT = mybir.dt.float32r if use_f32r else mybir.dt.float32

    def mm(x):
        return x.bitcast(MMT)

    # producers of matmul operands must write rounded f32r (walrus birverifier)
    r = mm

    s_chunks = seq // P          # 16  (128-row chunks)
    sb_chunks = seq // 512       # 4   (512-wide blocks)
    sk_chunks = seq // 1024      # 2   (1024-wide attention blocks)
    d_chunks = D // P            # 8
    dh_chunks = d_chunks // 2    # 4   (per d-half)
    j_chunks = JG // P           # 2

    nc = bacc.Bacc("TRN2", target_bir_lowering=False, debug=False,
                   num_devices=N_CORES)

    # x tensors are declared f32r: bit-identical to the f32 input data,
    # but marks every consumer chain as f32r for the walrus verifier (the PE
    # rounds f32r operands internally).
    x1b = nc.dram_tensor("x1b", [seq, D], MMT, kind="ExternalInput")
    x2b = nc.dram_tensor("x2b", [seq, D], MMT, kind="ExternalInput")
    wq = nc.dram_tensor("wq", [D, JG], F32, kind="ExternalInput")
    wk = nc.dram_tensor("wk", [D, JG], F32, kind="ExternalInput")
    wv = nc.dram_tensor("wv", [D, JG], F32, kind="ExternalInput")
    wo = nc.dram_tensor("wo", [JG, D], F32, kind="ExternalInput")
    bqr = nc.dram_tensor("bqr", [P, j_chunks], F32, kind="ExternalInput")
    bkr = nc.dram_tensor("bkr", [P, j_chunks], F32, kind="ExternalInput")
    y_out = nc.dram_tensor("y_out", [seq // GROUPS, D], F32,
                           kind="ExternalOutput")

    EXP = mybir.ActivationFunctionType.Exp
    DIV = mybir.AluOpType.divide

    with tile.TileContext(nc) as tc:
        with (
            tc.tile_pool(name="consts", bufs=1) as consts,
            tc.tile_pool(name="wqkv", bufs=3) as wqkv_pool,
            tc.tile_pool(name="wop", bufs=1) as wo_pool,
            tc.tile_pool(name="xload", bufs=3) as xload,
            tc.tile_pool(name="xt", bufs=2) as xt_pool,
            tc.tile_pool(name="acts", bufs=1) as acts,
            tc.tile_pool(name="ctp", bufs=2) as ct_pool,
            tc.tile_pool(name="epool", bufs=4) as epool,
            tc.tile_pool(name="small", bufs=2) as small,
            tc.tile_pool(name="ysb", bufs=4) as ysb,
            tc.tile_pool(name="psum_mm", bufs=2, space="PSUM") as psum_mm,
            tc.tile_pool(name="psum_s", bufs=(2 if exp_width == 1024 else 4), space="PSUM") as psum_s,
            tc.tile_pool(name="psum_u", bufs=2, space="PSUM") as psum_u,
            tc.tile_pool(name="dram", bufs=1, space="DRAM") as dram,
        ):
            # f32r identity: the moving operand's dtype prices the PE
            # transpose (f32r = 1.5 cycles/row vs 2.0 for f32); the bitcast
            # is bit-exact so there is no precision impact.
            ident_f32 = consts.tile([P, P], F32)
            make_identity(nc, ident_f32)
            ident = consts.tile([P, P], F32)
            nc.vector.tensor_copy(r(ident[:]), ident_f32[:])

            def load_weight_cast(wsb, w_dram, n_outer, width, pat):
                # DMA f32 chunks then cast into the f32r operand tile
                for o in range(n_outer):
                    st = ysb.tile([P, 1024], F32, tag="y",
                                  name=f"wst_{wsb.name}_{o}")
                    nc.sync.dma_start(
                        st[:, :width],
                        w_dram.rearrange(pat, p=P)[:, o, :])
                    nc.vector.tensor_copy(r(wsb[:, o, :]), st[:, :width])

            def load_slab(x_dram, sb):
                # two 1MB DMAs per slab; tile q-pair layout [P, 2, D]
                pairs = []
                for g in range(2):
                    xt_ = xload.tile([P, 2, D], MMT, tag="xload")
                    nc.sync.dma_start(
                        xt_[:],
                        x_dram[(sb * 4 + 2 * g) * P:(sb * 4 + 2 * g + 2) * P,
                               :].rearrange("(q p) d -> p q d", p=P))
                    pairs.append(xt_)
                return [pairs[q // 2][:, q % 2] for q in range(4)]

            def transpose_slab(x_dram, sb, use_act=False, xts=None):
                # x rows [sb*512, (sb+1)*512) x full D -> xT [P, d_chunks, 512]
                # (d-major). In phase A (use_act) the idle 2-bank score slots
                # hold 8 batched PE transposes evicted by ONE [128,1024] copy,
                # alternating ACT/DVE; during attention (x1) fall back to
                # single-bank "mm" tiles so the score slots stay free.
                if xts is None:
                    xts = load_slab(x_dram, sb)
                xT = xt_pool.tile([P, d_chunks, 512], F32, tag="xT")
                if use_act:
                    for dg in range(d_chunks // 2):
                        pt = psum_s.tile([P, 1024], F32, tag="s",
                                         name=f"ptx_{x_dram.name}_{sb}_{dg}")
                        for i in range(2):
                            dc = 2 * dg + i
                            for q in range(4):
                                nc.tensor.transpose(
                                    r(pt[:, i * 512 + q * P:
                                         i * 512 + (q + 1) * P]),
                                    mm(xts[q][:, dc * P:(dc + 1) * P]),
                                    mm(ident[:]))
                        out2 = xT[:, 2 * dg:2 * dg + 2, :]
                        if dg % 2 == 1:
                            nc.scalar.copy(r(out2), pt[:])
                        else:
                            nc.vector.tensor_copy(r(out2), pt[:])
                else:
                    for dc in range(d_chunks):
                        pt = psum_mm.tile([P, 512], F32, tag="mm")
                        for q in range(4):
                            nc.tensor.transpose(
                                r(pt[:, q * P:(q + 1) * P]),
                                mm(xts[q][:, dc * P:(dc + 1) * P]), mm(ident[:]))
                        nc.vector.tensor_copy(r(xT[:, dc, :]), pt[:])
                return xT

            # x2 slab 0 loads go first so transposes start immediately;
            # weight DMAs stream in behind them
            x2tiles0 = load_slab(x2b, 0)

            # qkv weights rotate through 2 shared slots (k, v, then q)
            wk_sb = wqkv_pool.tile([P, d_chunks, JG], F32, tag="wqkv")
            wv_sb = wqkv_pool.tile([P, d_chunks, JG], F32, tag="wqkv")
            wo_sb = wo_pool.tile([P, j_chunks, D], F32, tag="wo")
            load_weight_cast(wk_sb, wk, d_chunks, JG, "(o p) j -> p o j")
            load_weight_cast(wv_sb, wv, d_chunks, JG, "(o p) j -> p o j")
            load_weight_cast(wo_sb, wo, j_chunks, D, "(o p) n -> p o n")
            bq_sb = consts.tile([P, j_chunks], F32, tag="bq")
            bk_sb = consts.tile([P, j_chunks], F32, tag="bk")
            nc.sync.dma_start(bq_sb[:], bqr[:])
            nc.sync.dma_start(bk_sb[:], bkr[:])


            kT = acts.tile([P, j_chunks, seq], F32, tag="kT")
            qT = acts.tile([P, j_chunks, seq], F32, tag="qT")
            # V'' per head-column-block: cols 0..63 V_h, 64..127 ones
            vpp = acts.tile([P, s_chunks, HPC * P], F32, tag="vpp")

            ones_f32 = consts.tile([P, DH], F32, tag="ones")
            nc.vector.memset(ones_f32[:], 1.0)
            for si in range(s_chunks):
                ones_view = vpp[:, si].rearrange("p (h q) -> p h q", q=P)[:, :, DH:P]
                nc.vector.tensor_copy(
                    r(ones_view),
                    ones_f32[:, None, :].to_broadcast([P, HPC, DH]))

            def project_jmajor(xT_s, w_sb, sb, out, bias, use_act=False,
                               on_s=False):
                # out[j, sb-slab] = w.T @ xT_s + bias. on_s borrows the
                # attention score PSUM banks (idle before the first exp) so
                # projections pipeline in parallel with the next slab's
                # transposes instead of contending for the 2 "mm" slots.
                ssl = slice(sb * 512, (sb + 1) * 512)
                for jc in range(j_chunks):
                    if on_s:
                        pk = psum_s.tile([P, 512], F32, tag="s",
                                         name=f"pk_{w_sb.name}_{sb}_{jc}")
                    else:
                        pk = psum_mm.tile([P, 512], F32, tag="mm")
                    for dc in range(d_chunks):
                        nc.tensor.matmul(
                            pk[:],
                            mm(w_sb[:, dc, jc * P:(jc + 1) * P]),
                            mm(xT_s[:, dc, :]),
                            start=(dc == 0), stop=(dc == d_chunks - 1))
                    if use_act:
                        nc.scalar.add(r(out[:, jc, ssl]), pk[:],
                                      bias[:, jc:jc + 1])
                    else:
                        nc.vector.tensor_scalar_add(
                            r(out[:, jc, ssl]), pk[:], bias[:, jc:jc + 1])

            def project_v(xT_s, sb):
                # V[s-slab, j] = x2_slab @ Wv into the vpp head blocks
                for q in range(4):
                    si = sb * 4 + q
                    pv = psum_u.tile([P, JG], F32, tag="u")
                    for dc in range(d_chunks):
                        nc.tensor.matmul(
                            pv[:],
                            mm(xT_s[:, dc, q * P:(q + 1) * P]),
                            mm(wv_sb[:, dc, :]),
                            start=(dc == 0), stop=(dc == d_chunks - 1))
                    vv = vpp[:, si].rearrange("p (h q) -> p h q", q=P)[:, :, 0:DH]
                    nc.vector.tensor_copy(
                        r(vv), pv[:].rearrange("p (h q) -> p h q", q=DH))

            ybounce = dram.tile([seq, D], F32, tag="yin")

            cts = {}
            pus_by = {}

            def emit_oproj(sc, cT):
                for s8 in range(8):
                  with nc.named_scope("oproj"):
                    si = sc * 8 + s8
                    yt = ysb.tile([P, D], F32, tag="y",
                                  name=f"yt_{sc}_{s8}")
                    last = sc == sk_chunks - 1
                    for nck in range(2):
                        if last and (s8 * 2 + nck) % 2 == 1:
                            py = psum_s.tile([P, 512], F32, tag="s",
                                             name=f"py_{sc}_{s8}_{nck}")
                        else:
                            py = psum_mm.tile([P, 512], F32, tag="mm",
                                              name=f"py_{sc}_{s8}_{nck}")
                        for jc in range(j_chunks):
                            nc.tensor.matmul(
                                py[:],
                                mm(cT[:, jc, s8 * P:(s8 + 1) * P]),
                                mm(wo_sb[:, jc, nck * 512:(nck + 1) * 512]),
                                start=(jc == 0), stop=(jc == j_chunks - 1))
                        if last:
                            nc.scalar.copy(
                                yt[:, nck * 512:(nck + 1) * 512], py[:])
                        else:
                            nc.vector.tensor_copy(
                                yt[:, nck * 512:(nck + 1) * 512], py[:])
                    nc.sync.dma_start(ybounce[si * P:(si + 1) * P, :], yt[:])

            def emit_pv(sc, h, kc, et):
                jc, po = h // 2, (h % 2) * DH
                if kc == 0:
                    pus_by[(sc, h)] = [
                        psum_u.tile([P, 512], F32, tag="u",
                                    name=f"pu_{sc}_{h}_{i}")
                        for i in range(2)]
                pus = pus_by[(sc, h)]
                for half in range(2):
                    fsl = slice(half * 512, (half + 1) * 512)
                    nc.tensor.matmul(
                        pus[half][:],
                        mm(vpp[:, kc, h * P:(h + 1) * P]),
                        mm(et[:, fsl]),
                        start=(kc == 0), stop=(kc == s_chunks - 1))
                if kc == s_chunks - 1:
                    cT = cts[sc]
                    for half in range(2):
                        fsl = slice(half * 512, (half + 1) * 512)
                        rt = small.tile([DH, 512], F32, tag="rt",
                                        name=f"rt_{sc}_{h}_{half}")
                        nc.vector.reciprocal(rt[:], pus[half][DH:P, :])
                        nc.vector.tensor_mul(
                            r(cT[po:po + DH, jc, fsl]),
                            pus[half][0:DH, :], rt[:])
                    del pus_by[(sc, h)]
                    if h == HPC - 1:
                        emit_oproj(sc, cT)

            pend = []

            def emit_attn_unit(sc, h, kc):
              with nc.named_scope("attn"):
                if (h, kc) == (0, 0):
                    cts[sc] = ct_pool.tile([P, j_chunks, 1024], F32,
                                           tag="cT", name=f"cT_{sc}")
                jc, po = h // 2, (h % 2) * DH
                ps = psum_s.tile([P, 1024], F32, tag="s",
                                 name=f"ps_{sc}_{h}_{kc}")
                for half in range(2):
                    hsl = slice(sc * 1024 + half * 512,
                                sc * 1024 + (half + 1) * 512)
                    nc.tensor.matmul(
                        ps[:, half * 512:(half + 1) * 512],
                        mm(kT[po:po + DH, jc, kc * P:(kc + 1) * P]),
                        mm(qT[po:po + DH, jc, hsl]),
                        start=True, stop=True)
                et = epool.tile([P, 1024], F32, tag="e",
                                name=f"et_{sc}_{h}_{kc}")
                nc.scalar.activation(r(et[:]), ps[:], EXP, scale=0.125)
                pend.append((sc, h, kc, et))
                if len(pend) > 3:
                    emit_pv(*pend.pop(0))

            # ---- x2 -> K^T, V'' (per 512-row slab) ----
            for sb in range(sb_chunks):
                with nc.named_scope("x2t"):
                    x2T_s = transpose_slab(x2b, sb, use_act=True,
                                           xts=(x2tiles0 if sb == 0 else None))
                with nc.named_scope("kproj"):
                    project_jmajor(x2T_s, wk_sb, sb, kT, bk_sb, use_act=True)
                with nc.named_scope("vproj"):
                    project_v(x2T_s, sb)

            # ---- x1 -> Q^T (per slab; overlaps with attention below) ----
            wq_sb = wqkv_pool.tile([P, d_chunks, JG], F32, tag="wqkv")
            load_weight_cast(wq_sb, wq, d_chunks, JG, "(o p) j -> p o j")
            for sb in range(sb_chunks):
                with nc.named_scope("x1t"):
                    x1T_s = transpose_slab(x1b, sb)
                with nc.named_scope("qproj"):
                    project_jmajor(x1T_s, wq_sb, sb, qT, bq_sb, on_s=(sb < 2))

            # ---- attention units (flat, PV lagging exp by 2) ----
            for sc in range(sk_chunks):
                for h in range(HPC):
                    for kc in range(s_chunks):
                        emit_attn_unit(sc, h, kc)
            with nc.named_scope("attn"):
                for args in pend:
                    emit_pv(*args)

            # ---- sum partials across the 4 cores of this batch ----
            # Two half-sized ReduceScatters: the first depends only on the
            # first 1024 rows (written when attention chunk 0's out-projection
            # lands), so it overlaps chunk 1's attention instead of
            # serializing after all compute.
            if with_collective:
                half = seq // 2                 # 1024 rows per collective
                qr = seq // GROUPS // 2         # 256 rows per rank per half
                for ci in range(2):
                    ysc = dram.tile([qr, D], F32, tag="yout",
                                    name=f"ysc_{ci}")
                    nc.gpsimd.collective_compute(
                        "ReduceScatter",
                        mybir.AluOpType.add,
                        replica_groups=[[0, 1, 2, 3], [4, 5, 6, 7]],
                        ins=[ybounce[ci * half:(ci + 1) * half, :].opt()],
                        outs=[ysc[:].opt()],
                    )
                    nc.sync.dma_start(y_out[ci * qr:(ci + 1) * qr, :], ysc[:])
            else:
                nc.sync.dma_start(y_out[:], ybounce[:seq // GROUPS, :])

    nc.compile()
    return nc


def _get_program(seq=SEQ, use_f32r=True):
    key = (seq, use_f32r)
    if key not in _cached:
        _cached[key] = _build_program(seq, use_f32r)
    return _cached[key]


def make_in_maps(x1, x2, Wq, bq, Wk, bk, Wv, bv, Wo, bo):
    """Per-core input dicts for the SPMD program."""
    in_maps = []
    for c in range(N_CORES):
        b, g = c // GROUPS, c % GROUPS
        js = slice(g * JG, (g + 1) * JG)
        in_maps.append({
            "x1b": np.ascontiguousarray(x1[b]),
            "x2b": np.ascontiguousarray(x2[b]),
            "wq": np.ascontiguousarray(Wq[:, js]),
            "wk": np.ascontiguousarray(Wk[:, js]),
            "wv": np.ascontiguousarray(Wv[:, js]),
            "wo": np.ascontiguousarray(Wo[js, :]),
            "bqr": np.ascontiguousarray(bq[js].reshape(2, P).T),
            "bkr": np.ascontiguousarray(bk[js].reshape(2, P).T),
        })
    return in_maps


def assemble(results, Wv_bias_fix):
    """results: list of per-core {'y_out': [seq//GROUPS, D]}.

    y_out rows [0:q) = rank's quarter of input rows [0:seq/2);
    rows [q:2q) = rank's quarter of input rows [seq/2:seq)."""
    seq = results[0]["y_out"].shape[0] * GROUPS
    q = seq // GROUPS // 2
    Y = np.empty((B, seq, D), np.float32)
    for c in range(N_CORES):
        b, rr = c // GROUPS, c % GROUPS
        yo = results[c]["y_out"]
        Y[b, rr * q:(rr + 1) * q, :] = yo[:q]
        Y[b, seq // 2 + rr * q:seq // 2 + (rr + 1) * q, :] = yo[q:]
    Y += Wv_bias_fix
    return Y


def kernel(x1, x2, Wq, bq, Wk, bk, Wv, bv, Wo, bo):
    from concourse.bass_utils import run_bass_kernel_spmd

    x1 = np.asarray(x1, np.float32)
    x2 = np.asarray(x2, np.float32)
    Wq, bq = np.asarray(Wq, np.float32), np.asarray(bq, np.float32)
    Wk, bk = np.asarray(Wk, np.float32), np.asarray(bk, np.float32)
    Wv, bv = np.asarray(Wv, np.float32), np.asarray(bv, np.float32)
    Wo, bo = np.asarray(Wo, np.float32), np.asarray(bo, np.float32)

    nc = _get_program(SEQ)
    in_maps = make_in_maps(x1, x2, Wq, bq, Wk, bk, Wv, bv, Wo, bo)
    res = run_bass_kernel_spmd(nc, in_maps, core_ids=list(range(N_CORES)))
    fix = (bv @ Wo + bo).astype(np.float32)
    return assemble(res.results, fix)



# revision 9
# speedup vs baseline: 1.1825x; 1.1608x over previous
"""Multi-head cross-attention on 8 Trainium2 NeuronCores.

Sharding: data-parallel over batch (2) x tensor-parallel over heads (4 groups
of 4 heads). Core c handles batch c//4, head-group c%4 (a 256-wide slice of
the QKV projection space). Each core computes a partial output-projection
Y_partial = ctx_c @ Wo_c; a ReduceScatter(add) over each batch's 4 cores
leaves each core with a 512-row shard of the summed output, which the host
concatenates.

On-core dataflow:
  - x1/x2 arrive as bf16 (host-cast); x^T is produced by the DMA xbar
    (dma_start_transpose, 16x128 tiles) straight from DRAM -- the PE does no
    transposes at all. QKV projections run bf16 x bf16 into f32 PSUM.
  - Q^T/K^T = W.T @ x^T come out j-major, V = x @ Wv comes out s-major --
    exactly the operand layouts the attention matmuls need.
  - attention runs in f32r at full PE rate, tiled as (512-query chunk sc,
    head h, key-chunk pair): scores for two 128-key chunks land in one
    [128,1024] PSUM tile and are exp'd in a single scalar-engine op (no max
    subtraction: logits ~ N(0,1)). V carries 64 ones-columns so the softmax
    denominator accumulates in PSUM partitions 64..127 of the same PV
    matmul chain; one reciprocal+multiply normalizes into cT.
  - the PV stream lags the exp stream by a few units, and the next chunk's
    Q-projection plus the previous chunk's out-projection are emitted inside
    the attention stream so the PE never starves while the scalar engine
    works through the exps.
  - bq/bk are applied on-device (per-partition bias in j-major layout).
    bv/bo commute through softmax/out-projection exactly (softmax rows sum
    to 1), so the host adds bv @ Wo + bo to the final output.
  - a zero-matmul warms the PE p-state ramp during the initial DMA fill.
"""

import numpy as np

B, SEQ, D, H, DH = 2, 2048, 1024, 16, 64
N_CORES = 8
GROUPS = 4            # head-groups per batch (cores per batch)
JG = D // GROUPS      # 256 projection dims per core
HPC = H // GROUPS     # 4 heads per core
P = 128

_cached = {}


def _build_program(seq=SEQ, with_collective=True, lag=3):
    import concourse.tile as tile
    from concourse import bacc, mybir

    F32 = mybir.dt.float32
    BF16 = mybir.dt.bfloat16
    F32R = mybir.dt.float32r

    def r(x):
        return x.bitcast(F32R)

    mm = r  # matmul operands are f32r views of f32 tiles

    d_chunks = D // P            # 8
    j_chunks = JG // P           # 2
    n_slabs = seq // 512         # 4 (512-row x blocks and 512-query chunks)
    s_chunks = seq // P          # 16 (128-key chunks)
    n_kcp = s_chunks // 2        # 8 key-chunk pairs per (sc, h)

    nc = bacc.Bacc("TRN2", target_bir_lowering=False, debug=False,
                   num_devices=N_CORES)

    x1r = nc.dram_tensor("x1r", [seq, D], BF16, kind="ExternalInput")
    x2r = nc.dram_tensor("x2r", [seq, D], BF16, kind="ExternalInput")
    wq = nc.dram_tensor("wq", [D, JG], BF16, kind="ExternalInput")
    wk = nc.dram_tensor("wk", [D, JG], BF16, kind="ExternalInput")
    wv = nc.dram_tensor("wv", [D, JG], BF16, kind="ExternalInput")
    wo = nc.dram_tensor("wo", [JG, D], F32, kind="ExternalInput")
    bqr = nc.dram_tensor("bqr", [P, j_chunks], F32, kind="ExternalInput")
    bkr = nc.dram_tensor("bkr", [P, j_chunks], F32, kind="ExternalInput")
    y_out = nc.dram_tensor("y_out", [seq // GROUPS, D], F32,
                           kind="ExternalOutput")

    EXP = mybir.ActivationFunctionType.Exp

    with tile.TileContext(nc) as tc:
        with (
            tc.tile_pool(name="consts", bufs=1) as consts,
            tc.tile_pool(name="wqkv", bufs=3) as wqkv_pool,
            tc.tile_pool(name="wop", bufs=1) as wo_pool,
            tc.tile_pool(name="xt", bufs=5) as xt_pool,
            tc.tile_pool(name="acts", bufs=1) as acts,
            tc.tile_pool(name="ctp", bufs=2) as ct_pool,
            tc.tile_pool(name="epool", bufs=4) as epool,
            tc.tile_pool(name="small", bufs=2) as small,
            tc.tile_pool(name="ysb", bufs=4) as ysb,
            tc.tile_pool(name="psum_mm", bufs=2, space="PSUM") as psum_mm,
            tc.tile_pool(name="psum_s", bufs=2, space="PSUM") as psum_s,
            tc.tile_pool(name="psum_u", bufs=2, space="PSUM") as psum_u,
            tc.tile_pool(name="dram", bufs=1, space="DRAM") as dram,
        ):
            # PE p-state warmup: one dummy matmul as early as possible so the
            # 3us clock ramp elapses during the initial DMA fill.
            zt = consts.tile([P, P], BF16, tag="warm")
            nc.gpsimd.memset(zt[:], 0.0)
            pwarm = psum_mm.tile([P, 512], F32, tag="mm", name="pwarm")
            nc.tensor.matmul(pwarm[:, 0:16], zt[:], zt[:, 0:16],
                             start=True, stop=True)

            def xpose(dst, x_dram, sb):
                # x rows [sb*512,(sb+1)*512) -> dst[:, dc, :] = slab^T (bf16)
                for dc in range(d_chunks):
                    nc.sync.dma_start_transpose(
                        dst[:, dc, :],
                        x_dram[sb * 512:(sb + 1) * 512,
                               dc * P:(dc + 1) * P])

            # -- DMA order: x2 slab0 first so kproj starts ASAP --
            x2Ts = [xt_pool.tile([P, d_chunks, 512], BF16, tag="xT",
                                 name=f"x2T_{sb}") for sb in range(n_slabs)]
            xpose(x2Ts[0], x2r, 0)

            wk_sb = wqkv_pool.tile([P, d_chunks, JG], BF16, tag="wqkv")
            wv_sb = wqkv_pool.tile([P, d_chunks, JG], BF16, tag="wqkv")
            wq_sb = wqkv_pool.tile([P, d_chunks, JG], BF16, tag="wqkv")
            nc.sync.dma_start(wk_sb[:],
                              wk.rearrange("(o p) j -> p o j", p=P))
            nc.sync.dma_start(wv_sb[:],
                              wv.rearrange("(o p) j -> p o j", p=P))
            bq_sb = consts.tile([P, j_chunks], F32, tag="bq")
            bk_sb = consts.tile([P, j_chunks], F32, tag="bk")
            nc.sync.dma_start(bq_sb[:], bqr[:])
            nc.sync.dma_start(bk_sb[:], bkr[:])
            for sb in range(1, n_slabs):
                xpose(x2Ts[sb], x2r, sb)
            nc.sync.dma_start(wq_sb[:],
                              wq.rearrange("(o p) j -> p o j", p=P))
            wo_sb = wo_pool.tile([P, j_chunks, D], F32, tag="wo")
            for o in range(j_chunks):
                st = ysb.tile([P, D], F32, tag="y", name=f"wst_{o}")
                nc.sync.dma_start(
                    st[:], wo.rearrange("(o p) n -> p o n", p=P)[:, o, :])
                nc.vector.tensor_copy(r(wo_sb[:, o, :]), st[:])
            x1Ts = [xt_pool.tile([P, d_chunks, 512], BF16, tag="xT",
                                 name=f"x1T_{sb}") for sb in range(n_slabs)]
            xpose(x1Ts[0], x1r, 0)

            # -- persistent activations --
            kT = acts.tile([P, j_chunks, seq], F32, tag="kT")
            qT = acts.tile([P, j_chunks, seq], F32, tag="qT")
            # V'' per head-column-block: cols 0..63 V_h, 64..127 ones
            vpp = acts.tile([P, s_chunks, HPC * P], F32, tag="vpp")

            ones_f32 = consts.tile([P, DH], F32, tag="ones")
            nc.vector.memset(ones_f32[:], 1.0)
            for si in range(s_chunks):
                ones_view = vpp[:, si].rearrange(
                    "p (h q) -> p h q", q=P)[:, :, DH:P]
                # scalar engine is idle before attention; it also rounds f32r
                nc.scalar.copy(
                    r(ones_view),
                    ones_f32[:, None, :].to_broadcast([P, HPC, DH]))

            def project_jmajor(xT_s, w_sb, sb, out, bias):
                # out[:, jc, sb-slab] = w.T @ x^T + bias (j-major)
                for jc in range(j_chunks):
                    pk = psum_mm.tile([P, 512], F32, tag="mm")
                    for dc in range(d_chunks):
                        nc.tensor.matmul(
                            pk[:],
                            w_sb[:, dc, jc * P:(jc + 1) * P],
                            xT_s[:, dc, :],
                            start=(dc == 0), stop=(dc == d_chunks - 1))
                    nc.vector.tensor_scalar_add(
                        r(out[:, jc, sb * 512:(sb + 1) * 512]),
                        pk[:], bias[:, jc:jc + 1])

            def project_v(xT_s, sb):
                # V[s-slab, :] = x2_slab @ Wv into the vpp head blocks
                for q in range(4):
                    si = sb * 4 + q
                    pv = psum_u.tile([P, 512], F32, tag="u")
                    for dc in range(d_chunks):
                        nc.tensor.matmul(
                            pv[:, 0:JG],
                            xT_s[:, dc, q * P:(q + 1) * P],
                            wv_sb[:, dc, :],
                            start=(dc == 0), stop=(dc == d_chunks - 1))
                    vv = vpp[:, si].rearrange(
                        "p (h q) -> p h q", q=P)[:, :, 0:DH]
                    nc.vector.tensor_copy(
                        r(vv),
                        pv[:, 0:JG].rearrange("p (h q) -> p h q", q=DH))

            # -- x2 -> K^T, V''; x1 transposes stream behind on the DMA --
            for sb in range(n_slabs):
                with nc.named_scope("kproj"):
                    project_jmajor(x2Ts[sb], wk_sb, sb, kT, bk_sb)
                with nc.named_scope("vproj"):
                    project_v(x2Ts[sb], sb)
                if sb >= 1:
                    # x1T slab sb reuses x2T slab sb-1's pool slot; emit its
                    # DMA only after that slab's readers (kproj/vproj above)
                    xpose(x1Ts[sb], x1r, sb)
            with nc.named_scope("qproj"):
                project_jmajor(x1Ts[0], wq_sb, 0, qT, bq_sb)

            ybounce = dram.tile([seq, D], F32, tag="yin")

            cts = {}
            pus = {}

            def emit_oproj(sc, cT):
                for s8 in range(4):
                  with nc.named_scope("oproj"):
                    yt = ysb.tile([P, D], F32, tag="y",
                                  name=f"yt_{sc}_{s8}")
                    for nck in range(2):
                        py = psum_mm.tile([P, 512], F32, tag="mm",
                                          name=f"py_{sc}_{s8}_{nck}")
                        for jc in range(j_chunks):
                            nc.tensor.matmul(
                                py[:],
                                mm(cT[:, jc, s8 * P:(s8 + 1) * P]),
                                mm(wo_sb[:, jc, nck * 512:(nck + 1) * 512]),
                                start=(jc == 0), stop=(jc == j_chunks - 1))
                        nc.vector.tensor_copy(
                            yt[:, nck * 512:(nck + 1) * 512], py[:])
                    si = sc * 4 + s8
                    nc.sync.dma_start(ybounce[si * P:(si + 1) * P, :], yt[:])

            def emit_pv(sc, h, kcp, et):
              with nc.named_scope("attn"):
                jc, po = h // 2, (h % 2) * DH
                if kcp == 0:
                    pus[(sc, h)] = psum_u.tile([P, 512], F32, tag="u",
                                               name=f"pu_{sc}_{h}")
                pu = pus[(sc, h)]
                for dk in range(2):
                    kc = kcp * 2 + dk
                    nc.tensor.matmul(
                        pu[:],
                        mm(vpp[:, kc, h * P:(h + 1) * P]),
                        mm(et[:, dk * 512:(dk + 1) * 512]),
                        start=(kcp == 0 and dk == 0),
                        stop=(kcp == n_kcp - 1 and dk == 1))
                if kcp == n_kcp - 1:
                    cT = cts[sc]
                    rt = small.tile([DH, 512], F32, tag="rt",
                                    name=f"rt_{sc}_{h}")
                    nc.vector.reciprocal(rt[:], pu[DH:P, :])
                    nc.vector.tensor_mul(
                        r(cT[po:po + DH, jc, :]), pu[0:DH, :], rt[:])
                    del pus[(sc, h)]
                    if h == HPC - 1:
                        emit_oproj(sc, cts.pop(sc))

            pend = []

            def emit_attn_unit(sc, h, kcp):
              with nc.named_scope("attn"):
                if (h, kcp) == (0, 0):
                    cts[sc] = ct_pool.tile([P, j_chunks, 512], F32,
                                           tag="cT", name=f"cT_{sc}")
                jc, po = h // 2, (h % 2) * DH
                ps = psum_s.tile([P, 1024], F32, tag="s",
                                 name=f"ps_{sc}_{h}_{kcp}")
                for dk in range(2):
                    kc = kcp * 2 + dk
                    nc.tensor.matmul(
                        ps[:, dk * 512:(dk + 1) * 512],
                        mm(kT[po:po + DH, jc, kc * P:(kc + 1) * P]),
                        mm(qT[po:po + DH, jc, sc * 512:(sc + 1) * 512]),
                        start=True, stop=True)
                et = epool.tile([P, 1024], F32, tag="e",
                                name=f"et_{sc}_{h}_{kcp}")
                nc.scalar.activation(r(et[:]), ps[:], EXP, scale=0.125)
                pend.append((sc, h, kcp, et))
                if len(pend) > lag:
                    emit_pv(*pend.pop(0))

            # -- attention: 4 chunks of 512 queries; qproj for the next
            #    chunk is emitted inside the stream to keep the PE fed --
            for sc in range(n_slabs):
                for h in range(HPC):
                    if h == 2 and sc + 1 < n_slabs:
                        with nc.named_scope("qproj"):
                            project_jmajor(x1Ts[sc + 1], wq_sb, sc + 1,
                                           qT, bq_sb)
                    for kcp in range(n_kcp):
                        emit_attn_unit(sc, h, kcp)
            with nc.named_scope("attn"):
                for args in pend:
                    emit_pv(*args)

            # -- sum partials across the 4 cores of this batch --
            # Two half-sized ReduceScatters: the first depends only on the
            # first 1024 rows, so it overlaps the second half's attention.
            if with_collective:
                half = seq // 2                 # 1024 rows per collective
                qr = seq // GROUPS // 2         # 256 rows per rank per half
                for ci in range(2):
                    ysc = dram.tile([qr, D], F32, tag="yout",
                                    name=f"ysc_{ci}")
                    nc.gpsimd.collective_compute(
                        "ReduceScatter",
                        mybir.AluOpType.add,
                        replica_groups=[[0, 1, 2, 3], [4, 5, 6, 7]],
                        ins=[ybounce[ci * half:(ci + 1) * half, :].opt()],
                        outs=[ysc[:].opt()],
                    )
                    nc.sync.dma_start(y_out[ci * qr:(ci + 1) * qr, :], ysc[:])
            else:
                nc.sync.dma_start(y_out[:], ybounce[:seq // GROUPS, :])

    nc.compile()
    return nc


def _get_program(seq=SEQ):
    if seq not in _cached:
        _cached[seq] = _build_program(seq)
    return _cached[seq]


def make_in_maps(x1, x2, Wq, bq, Wk, bk, Wv, bv, Wo, bo):
    """Per-core input dicts for the SPMD program (x and Wqkv host-cast to
    bf16; attention itself stays f32r on-chip)."""
    import ml_dtypes
    bf16 = ml_dtypes.bfloat16
    x1 = np.asarray(x1, np.float32).astype(bf16)
    x2 = np.asarray(x2, np.float32).astype(bf16)
    Wqh = np.asarray(Wq, np.float32).astype(bf16)
    Wkh = np.asarray(Wk, np.float32).astype(bf16)
    Wvh = np.asarray(Wv, np.float32).astype(bf16)
    Wo = np.asarray(Wo, np.float32)
    bq = np.asarray(bq, np.float32)
    bk = np.asarray(bk, np.float32)
    in_maps = []
    for c in range(N_CORES):
        b, g = c // GROUPS, c % GROUPS
        js = slice(g * JG, (g + 1) * JG)
        in_maps.append({
            "x1r": np.ascontiguousarray(x1[b]),
            "x2r": np.ascontiguousarray(x2[b]),
            "wq": np.ascontiguousarray(Wqh[:, js]),
            "wk": np.ascontiguousarray(Wkh[:, js]),
            "wv": np.ascontiguousarray(Wvh[:, js]),
            "wo": np.ascontiguousarray(Wo[js, :]),
            "bqr": np.ascontiguousarray(bq[js].reshape(2, P).T),
            "bkr": np.ascontiguousarray(bk[js].reshape(2, P).T),
        })
    return in_maps


def assemble(results, Wv_bias_fix):
    """results: list of per-core {'y_out': [seq//GROUPS, D]}.

    y_out rows [0:q) = rank's quarter of input rows [0:seq/2);
    rows [q:2q) = rank's quarter of input rows [seq/2:seq)."""
    seq = results[0]["y_out"].shape[0] * GROUPS
    q = seq // GROUPS // 2
    Y = np.empty((B, seq, D), np.float32)
    for c in range(N_CORES):
        b, rr = c // GROUPS, c % GROUPS
        yo = results[c]["y_out"]
        Y[b, rr * q:(rr + 1) * q, :] = yo[:q]
        Y[b, seq // 2 + rr * q:seq // 2 + (rr + 1) * q, :] = yo[q:]
    Y += Wv_bias_fix
    return Y


def kernel(x1, x2, Wq, bq, Wk, bk, Wv, bv, Wo, bo):
    from concourse.bass_utils import run_bass_kernel_spmd

    Wo = np.asarray(Wo, np.float32)
    bv = np.asarray(bv, np.float32)
    bo = np.asarray(bo, np.float32)

    nc = _get_program(SEQ)
    in_maps = make_in_maps(x1, x2, Wq, bq, Wk, bk, Wv, bv, Wo, bo)
    res = run_bass_kernel_spmd(nc, in_maps, core_ids=list(range(N_CORES)))
    fix = (bv @ Wo + bo).astype(np.float32)
    return assemble(res.results, fix)


# revision 16
# speedup vs baseline: 1.2159x; 1.0283x over previous
"""Multi-head cross-attention on 8 Trainium2 NeuronCores.

Sharding: data-parallel over batch (2) x tensor-parallel over heads (4 groups
of 4 heads). Core c handles batch c//4, head-group c%4 (a 256-wide slice of
the QKV projection space). Each core computes a partial output-projection
Y_partial = ctx_c @ Wo_c; a ReduceScatter(add) over each batch's 4 cores
leaves each core with a 512-row shard of the summed output, which the host
concatenates.

On-core dataflow:
  - x1/x2 arrive as bf16 (host-cast); x^T is produced by the DMA xbar
    (dma_start_transpose, 16x128 tiles) straight from DRAM -- the PE does no
    transposes at all. QKV projections run bf16 x bf16 into f32 PSUM.
  - Q^T/K^T = W.T @ x^T come out j-major, V = x @ Wv comes out s-major --
    exactly the operand layouts the attention matmuls need.
  - attention runs in f32r at full PE rate, tiled as (512-query chunk sc,
    head h, key-chunk pair): scores for two 128-key chunks land in one
    [128,1024] PSUM tile and are exp'd in a single scalar-engine op (no max
    subtraction: logits ~ N(0,1)). V carries 64 ones-columns so the softmax
    denominator accumulates in PSUM partitions 64..127 of the same PV
    matmul chain; one reciprocal+multiply normalizes into cT.
  - the PV stream lags the exp stream by a few units, and the next chunk's
    Q-projection plus the previous chunk's out-projection are emitted inside
    the attention stream so the PE never starves while the scalar engine
    works through the exps.
  - bq/bk are applied on-device (per-partition bias in j-major layout).
    bv/bo commute through softmax/out-projection exactly (softmax rows sum
    to 1), so the host adds bv @ Wo + bo to the final output.
  - a zero-matmul warms the PE p-state ramp during the initial DMA fill.
"""

import numpy as np

B, SEQ, D, H, DH = 2, 2048, 1024, 16, 64
N_CORES = 8
GROUPS = 4            # head-groups per batch (cores per batch)
JG = D // GROUPS      # 256 projection dims per core
HPC = H // GROUPS     # 4 heads per core
P = 128

_cached = {}


def _build_program(seq=SEQ, with_collective=True, lag=3):
    import concourse.tile as tile
    from concourse import bacc, mybir

    F32 = mybir.dt.float32
    BF16 = mybir.dt.bfloat16
    F32R = mybir.dt.float32r

    def r(x):
        return x.bitcast(F32R)

    mm = r  # matmul operands are f32r views of f32 tiles

    d_chunks = D // P            # 8
    j_chunks = JG // P           # 2
    n_slabs = seq // 512         # 4 (512-row x blocks and 512-query chunks)
    s_chunks = seq // P          # 16 (128-key chunks)
    n_kcp = s_chunks // 2        # 8 key-chunk pairs per (sc, h)

    nc = bacc.Bacc("TRN2", target_bir_lowering=False, debug=False,
                   num_devices=N_CORES)

    x1r = nc.dram_tensor("x1r", [seq, D], BF16, kind="ExternalInput")
    x2r = nc.dram_tensor("x2r", [seq, D], BF16, kind="ExternalInput")
    wq = nc.dram_tensor("wq", [D, JG], BF16, kind="ExternalInput")
    wk = nc.dram_tensor("wk", [D, JG], BF16, kind="ExternalInput")
    wv = nc.dram_tensor("wv", [D, JG], BF16, kind="ExternalInput")
    wo = nc.dram_tensor("wo", [JG, D], F32, kind="ExternalInput")
    bqr = nc.dram_tensor("bqr", [P, j_chunks], F32, kind="ExternalInput")
    bkr = nc.dram_tensor("bkr", [P, j_chunks], F32, kind="ExternalInput")
    y_out = nc.dram_tensor("y_out", [seq // GROUPS, D], F32,
                           kind="ExternalOutput")

    EXP = mybir.ActivationFunctionType.Exp

    with tile.TileContext(nc) as tc:
        with (
            tc.tile_pool(name="consts", bufs=1) as consts,
            tc.tile_pool(name="wqkv", bufs=3) as wqkv_pool,
            tc.tile_pool(name="wop", bufs=1) as wo_pool,
            tc.tile_pool(name="xt", bufs=5) as xt_pool,
            tc.tile_pool(name="acts", bufs=1) as acts,
            tc.tile_pool(name="ctp", bufs=2) as ct_pool,
            tc.tile_pool(name="epool", bufs=4) as epool,
            tc.tile_pool(name="small", bufs=2) as small,
            tc.tile_pool(name="ysb", bufs=4) as ysb,
            tc.tile_pool(name="psum_mm", bufs=2, space="PSUM") as psum_mm,
            tc.tile_pool(name="psum_s", bufs=2, space="PSUM") as psum_s,
            tc.tile_pool(name="psum_u", bufs=2, space="PSUM") as psum_u,
            tc.tile_pool(name="dram", bufs=1, space="DRAM") as dram,
        ):
            # PE p-state warmup: dummy matmuls spread out by ping-ponging
            # through a DVE copy (two semaphore hops each, ~400ns apart) so
            # the tensor engine never idles long enough to reset its clock
            # ramp while the initial DMAs fill SBUF.
            zt = consts.tile([P, P], BF16, tag="warm")
            nc.gpsimd.memset(zt[:], 0.0)
            wsb = consts.tile([P, 16], F32, tag="warm2")
            pwarm = psum_mm.tile([P, 512], F32, tag="mm", name="pwarm")
            for _ in range(16):
                nc.tensor.matmul(pwarm[:, 0:16], zt[:], zt[:, 0:16],
                                 start=True, stop=True)
                nc.vector.tensor_copy(wsb[:], pwarm[:, 0:16])

            def xpose(dst, x_dram, sb):
                # x rows [sb*512,(sb+1)*512) -> dst[:, dc, :] = slab^T (bf16).
                # One xbar instruction transposes four 128-col blocks into the
                # 3D [128, 4, 512] layout directly (in [512, 4*128] reshaped
                # (512,4,128) then reversed-transposed is exactly d-major).
                for g in range(d_chunks // 4):
                    nc.sync.dma_start_transpose(
                        dst[:, 4 * g:4 * (g + 1), :],
                        x_dram[sb * 512:(sb + 1) * 512,
                               g * 512:(g + 1) * 512])

            # -- DMA order: wk first (first kproj needs it), then x2 slab0
            #    transposes so kproj starts ASAP --
            x2Ts = [xt_pool.tile([P, d_chunks, 512], BF16, tag="xT",
                                 name=f"x2T_{sb}") for sb in range(n_slabs)]
            wk_sb = wqkv_pool.tile([P, d_chunks, JG], BF16, tag="wqkv")
            wv_sb = wqkv_pool.tile([P, d_chunks, JG], BF16, tag="wqkv")
            wq_sb = wqkv_pool.tile([P, d_chunks, JG], BF16, tag="wqkv")
            nc.sync.dma_start(wk_sb[:],
                              wk.rearrange("(o p) j -> p o j", p=P))
            nc.sync.dma_start(wv_sb[:],
                              wv.rearrange("(o p) j -> p o j", p=P))
            xpose(x2Ts[0], x2r, 0)
            bq_sb = consts.tile([P, j_chunks], F32, tag="bq")
            bk_sb = consts.tile([P, j_chunks], F32, tag="bk")
            nc.sync.dma_start(bq_sb[:], bqr[:])
            nc.sync.dma_start(bk_sb[:], bkr[:])
            for sb in range(1, n_slabs):
                xpose(x2Ts[sb], x2r, sb)
            nc.sync.dma_start(wq_sb[:],
                              wq.rearrange("(o p) j -> p o j", p=P))
            wo_sb = wo_pool.tile([P, j_chunks, D], F32, tag="wo")
            for o in range(j_chunks):
                st = ysb.tile([P, D], F32, tag="y", name=f"wst_{o}")
                nc.sync.dma_start(
                    st[:], wo.rearrange("(o p) n -> p o n", p=P)[:, o, :])
                nc.vector.tensor_copy(r(wo_sb[:, o, :]), st[:])
            x1Ts = [xt_pool.tile([P, d_chunks, 512], BF16, tag="xT",
                                 name=f"x1T_{sb}") for sb in range(n_slabs)]
            xpose(x1Ts[0], x1r, 0)

            # -- persistent activations --
            kT = acts.tile([P, j_chunks, seq], F32, tag="kT")
            qT = acts.tile([P, j_chunks, seq], F32, tag="qT")
            # V'' per head-column-block: cols 0..63 V_h, 64..127 ones
            vpp = acts.tile([P, s_chunks, HPC * P], F32, tag="vpp")

            ones_f32 = consts.tile([P, DH], F32, tag="ones")
            nc.vector.memset(ones_f32[:], 1.0)
            for si in range(s_chunks):
                ones_view = vpp[:, si].rearrange(
                    "p (h q) -> p h q", q=P)[:, :, DH:P]
                # scalar engine is idle before attention; it also rounds f32r
                nc.scalar.copy(
                    r(ones_view),
                    ones_f32[:, None, :].to_broadcast([P, HPC, DH]))

            def project_jmajor(xT_s, w_sb, sb, out, bias):
                # out[:, jc, sb-slab] = w.T @ x^T + bias (j-major)
                for jc in range(j_chunks):
                    pk = psum_mm.tile([P, 512], F32, tag="mm")
                    for dc in range(d_chunks):
                        nc.tensor.matmul(
                            pk[:],
                            w_sb[:, dc, jc * P:(jc + 1) * P],
                            xT_s[:, dc, :],
                            start=(dc == 0), stop=(dc == d_chunks - 1))
                    nc.vector.tensor_scalar_add(
                        r(out[:, jc, sb * 512:(sb + 1) * 512]),
                        pk[:], bias[:, jc:jc + 1])

            def project_v(xT_s, sb):
                # V[s-slab, :] = x2_slab @ Wv into the vpp head blocks
                for q in range(4):
                    si = sb * 4 + q
                    pv = psum_u.tile([P, 512], F32, tag="u")
                    for dc in range(d_chunks):
                        nc.tensor.matmul(
                            pv[:, 0:JG],
                            xT_s[:, dc, q * P:(q + 1) * P],
                            wv_sb[:, dc, :],
                            start=(dc == 0), stop=(dc == d_chunks - 1))
                    vv = vpp[:, si].rearrange(
                        "p (h q) -> p h q", q=P)[:, :, 0:DH]
                    nc.vector.tensor_copy(
                        r(vv),
                        pv[:, 0:JG].rearrange("p (h q) -> p h q", q=DH))

            # -- x2 -> K^T, V''; x1 transposes stream behind on the DMA --
            for sb in range(n_slabs):
                with nc.named_scope("kproj"):
                    project_jmajor(x2Ts[sb], wk_sb, sb, kT, bk_sb)
                with nc.named_scope("vproj"):
                    project_v(x2Ts[sb], sb)
                if sb >= 1:
                    # x1T slab sb reuses x2T slab sb-1's pool slot; emit its
                    # DMA only after that slab's readers (kproj/vproj above)
                    xpose(x1Ts[sb], x1r, sb)
            with nc.named_scope("qproj"):
                project_jmajor(x1Ts[0], wq_sb, 0, qT, bq_sb)

            ybounce = dram.tile([seq, D], F32, tag="yin")

            cts = {}
            pus = {}

            def emit_oproj(sc, cT):
                for s8 in range(4):
                  with nc.named_scope("oproj"):
                    yt = ysb.tile([P, D], F32, tag="y",
                                  name=f"yt_{sc}_{s8}")
                    for nck in range(2):
                        py = psum_mm.tile([P, 512], F32, tag="mm",
                                          name=f"py_{sc}_{s8}_{nck}")
                        for jc in range(j_chunks):
                            nc.tensor.matmul(
                                py[:],
                                mm(cT[:, jc, s8 * P:(s8 + 1) * P]),
                                mm(wo_sb[:, jc, nck * 512:(nck + 1) * 512]),
                                start=(jc == 0), stop=(jc == j_chunks - 1))
                        nc.vector.tensor_copy(
                            yt[:, nck * 512:(nck + 1) * 512], py[:])
                    si = sc * 4 + s8
                    if with_collective or sc > 0:
                        nc.sync.dma_start(
                            ybounce[si * P:(si + 1) * P, :], yt[:])
                    else:
                        # timed (no-collective) build: the final DRAM->DRAM
                        # copy stands in for the untimed ReduceScatter, so
                        # write the covered rows straight to the output
                        nc.sync.dma_start(
                            y_out[si * P:(si + 1) * P, :], yt[:])

            def emit_pv(sc, h, kcp, et):
              with nc.named_scope("attn"):
                jc, po = h // 2, (h % 2) * DH
                if kcp == 0:
                    pus[(sc, h)] = psum_u.tile([P, 512], F32, tag="u",
                                               name=f"pu_{sc}_{h}")
                pu = pus[(sc, h)]
                for dk in range(2):
                    kc = kcp * 2 + dk
                    nc.tensor.matmul(
                        pu[:],
                        mm(vpp[:, kc, h * P:(h + 1) * P]),
                        mm(et[:, dk * 512:(dk + 1) * 512]),
                        start=(kcp == 0 and dk == 0),
                        stop=(kcp == n_kcp - 1 and dk == 1))
                if kcp == n_kcp - 1:
                    cT = cts[sc]
                    rt = small.tile([DH, 512], F32, tag="rt",
                                    name=f"rt_{sc}_{h}")
                    nc.vector.reciprocal(rt[:], pu[DH:P, :])
                    nc.vector.tensor_mul(
                        r(cT[po:po + DH, jc, :]), pu[0:DH, :], rt[:])
                    del pus[(sc, h)]
                    if h == HPC - 1:
                        emit_oproj(sc, cts.pop(sc))

            pend = []

            def emit_attn_unit(sc, h, kcp):
              with nc.named_scope("attn"):
                if (h, kcp) == (0, 0):
                    cts[sc] = ct_pool.tile([P, j_chunks, 512], F32,
                                           tag="cT", name=f"cT_{sc}")
                jc, po = h // 2, (h % 2) * DH
                ps = psum_s.tile([P, 1024], F32, tag="s",
                                 name=f"ps_{sc}_{h}_{kcp}")
                for dk in range(2):
                    kc = kcp * 2 + dk
                    nc.tensor.matmul(
                        ps[:, dk * 512:(dk + 1) * 512],
                        mm(kT[po:po + DH, jc, kc * P:(kc + 1) * P]),
                        mm(qT[po:po + DH, jc, sc * 512:(sc + 1) * 512]),
                        start=True, stop=True)
                et = epool.tile([P, 1024], F32, tag="e",
                                name=f"et_{sc}_{h}_{kcp}")
                nc.scalar.activation(r(et[:]), ps[:], EXP, scale=0.125)
                pend.append((sc, h, kcp, et))
                if len(pend) > lag:
                    emit_pv(*pend.pop(0))

            # -- attention: 4 chunks of 512 queries; qproj for the next
            #    chunk is emitted inside the stream to keep the PE fed --
            for sc in range(n_slabs):
                for h in range(HPC):
                    if h == 2 and sc + 1 < n_slabs:
                        with nc.named_scope("qproj"):
                            project_jmajor(x1Ts[sc + 1], wq_sb, sc + 1,
                                           qT, bq_sb)
                    for kcp in range(n_kcp):
                        emit_attn_unit(sc, h, kcp)
            with nc.named_scope("attn"):
                for args in pend:
                    emit_pv(*args)

            # -- sum partials across the 4 cores of this batch --
            # Two half-sized ReduceScatters: the first depends only on the
            # first 1024 rows, so it overlaps the second half's attention.
            if with_collective:
                half = seq // 2                 # 1024 rows per collective
                qr = seq // GROUPS // 2         # 256 rows per rank per half
                for ci in range(2):
                    ysc = dram.tile([qr, D], F32, tag="yout",
                                    name=f"ysc_{ci}")
                    nc.gpsimd.collective_compute(
                        "ReduceScatter",
                        mybir.AluOpType.add,
                        replica_groups=[[0, 1, 2, 3], [4, 5, 6, 7]],
                        ins=[ybounce[ci * half:(ci + 1) * half, :].opt()],
                        outs=[ysc[:].opt()],
                    )
                    nc.sync.dma_start(y_out[ci * qr:(ci + 1) * qr, :], ysc[:])
            # (no-collective build: y_out rows were written directly by
            # emit_oproj's sc==0 DMAs)

    nc.compile()
    return nc


def _get_program(seq=SEQ):
    if seq not in _cached:
        _cached[seq] = _build_program(seq)
    return _cached[seq]


def make_in_maps(x1, x2, Wq, bq, Wk, bk, Wv, bv, Wo, bo):
    """Per-core input dicts for the SPMD program (x and Wqkv host-cast to
    bf16; attention itself stays f32r on-chip)."""
    import ml_dtypes
    bf16 = ml_dtypes.bfloat16
    x1 = np.asarray(x1, np.float32).astype(bf16)
    x2 = np.asarray(x2, np.float32).astype(bf16)
    Wqh = np.asarray(Wq, np.float32).astype(bf16)
    Wkh = np.asarray(Wk, np.float32).astype(bf16)
    Wvh = np.asarray(Wv, np.float32).astype(bf16)
    Wo = np.asarray(Wo, np.float32)
    bq = np.asarray(bq, np.float32)
    bk = np.asarray(bk, np.float32)
    in_maps = []
    for c in range(N_CORES):
        b, g = c // GROUPS, c % GROUPS
        js = slice(g * JG, (g + 1) * JG)
        in_maps.append({
            "x1r": np.ascontiguousarray(x1[b]),
            "x2r": np.ascontiguousarray(x2[b]),
            "wq": np.ascontiguousarray(Wqh[:, js]),
            "wk": np.ascontiguousarray(Wkh[:, js]),
            "wv": np.ascontiguousarray(Wvh[:, js]),
            "wo": np.ascontiguousarray(Wo[js, :]),
            "bqr": np.ascontiguousarray(bq[js].reshape(2, P).T),
            "bkr": np.ascontiguousarray(bk[js].reshape(2, P).T),
        })
    return in_maps


def assemble(results, Wv_bias_fix):
    """results: list of per-core {'y_out': [seq//GROUPS, D]}.

    y_out rows [0:q) = rank's quarter of input rows [0:seq/2);
    rows [q:2q) = rank's quarter of input rows [seq/2:seq)."""
    seq = results[0]["y_out"].shape[0] * GROUPS
    q = seq // GROUPS // 2
    Y = np.empty((B, seq, D), np.float32)
    for c in range(N_CORES):
        b, rr = c // GROUPS, c % GROUPS
        yo = results[c]["y_out"]
        Y[b, rr * q:(rr + 1) * q, :] = yo[:q]
        Y[b, seq // 2 + rr * q:seq // 2 + (rr + 1) * q, :] = yo[q:]
    Y += Wv_bias_fix
    return Y


def kernel(x1, x2, Wq, bq, Wk, bk, Wv, bv, Wo, bo):
    from concourse.bass_utils import run_bass_kernel_spmd

    Wo = np.asarray(Wo, np.float32)
    bv = np.asarray(bv, np.float32)
    bo = np.asarray(bo, np.float32)

    nc = _get_program(SEQ)
    in_maps = make_in_maps(x1, x2, Wq, bq, Wk, bk, Wv, bv, Wo, bo)
    res = run_bass_kernel_spmd(nc, in_maps, core_ids=list(range(N_CORES)))
    fix = (bv @ Wo + bo).astype(np.float32)
    return assemble(res.results, fix)


# revision 20
# speedup vs baseline: 1.2428x; 1.0221x over previous
"""Multi-head cross-attention on 8 Trainium2 NeuronCores.

Sharding: data-parallel over batch (2) x tensor-parallel over heads (4 groups
of 4 heads). Core c handles batch c//4, head-group c%4 (a 256-wide slice of
the QKV projection space). Each core computes a partial output-projection
Y_partial = ctx_c @ Wo_c; a ReduceScatter(add) over each batch's 4 cores
leaves each core with a 512-row shard of the summed output, which the host
concatenates.

On-core dataflow:
  - x1/x2 arrive as bf16 (host-cast); x^T is produced by the DMA xbar
    (dma_start_transpose, 16x128 tiles) straight from DRAM -- the PE does no
    transposes at all. QKV projections run bf16 x bf16 into f32 PSUM.
  - Q^T/K^T = W.T @ x^T come out j-major, V = x @ Wv comes out s-major --
    exactly the operand layouts the attention matmuls need.
  - attention runs in f32r at full PE rate, tiled as (512-query chunk sc,
    head h, key-chunk pair): scores for two 128-key chunks land in one
    [128,1024] PSUM tile and are exp'd in a single scalar-engine op (no max
    subtraction: logits ~ N(0,1)). V carries 64 ones-columns so the softmax
    denominator accumulates in PSUM partitions 64..127 of the same PV
    matmul chain; one reciprocal+multiply normalizes into cT.
  - the PV stream lags the exp stream by a few units, and the next chunk's
    Q-projection plus the previous chunk's out-projection are emitted inside
    the attention stream so the PE never starves while the scalar engine
    works through the exps.
  - bq/bk are applied on-device (per-partition bias in j-major layout).
    bv/bo commute through softmax/out-projection exactly (softmax rows sum
    to 1), so the host adds bv @ Wo + bo to the final output.
  - a zero-matmul warms the PE p-state ramp during the initial DMA fill.
"""

import numpy as np

B, SEQ, D, H, DH = 2, 2048, 1024, 16, 64
N_CORES = 8
GROUPS = 4            # head-groups per batch (cores per batch)
JG = D // GROUPS      # 256 projection dims per core
HPC = H // GROUPS     # 4 heads per core
P = 128

_cached = {}


def _build_program(seq=SEQ, with_collective=True, lag=3):
    import concourse.tile as tile
    from concourse import bacc, mybir

    F32 = mybir.dt.float32
    BF16 = mybir.dt.bfloat16
    F32R = mybir.dt.float32r

    def r(x):
        return x.bitcast(F32R)

    mm = r  # matmul operands are f32r views of f32 tiles

    d_chunks = D // P            # 8
    j_chunks = JG // P           # 2
    n_slabs = seq // 512         # 4 (512-row x blocks and 512-query chunks)
    s_chunks = seq // P          # 16 (128-key chunks)
    n_kcp = s_chunks // 2        # 8 key-chunk pairs per (sc, h)

    nc = bacc.Bacc("TRN2", target_bir_lowering=False, debug=False,
                   num_devices=N_CORES)

    x1r = nc.dram_tensor("x1r", [seq, D], BF16, kind="ExternalInput")
    x2r = nc.dram_tensor("x2r", [seq, D], BF16, kind="ExternalInput")
    wq = nc.dram_tensor("wq", [D, JG], BF16, kind="ExternalInput")
    wk = nc.dram_tensor("wk", [D, JG], BF16, kind="ExternalInput")
    wv = nc.dram_tensor("wv", [D, JG], BF16, kind="ExternalInput")
    wo = nc.dram_tensor("wo", [JG, D], F32, kind="ExternalInput")
    bqr = nc.dram_tensor("bqr", [P, j_chunks], F32, kind="ExternalInput")
    bkr = nc.dram_tensor("bkr", [P, j_chunks], F32, kind="ExternalInput")
    y_out = nc.dram_tensor("y_out", [seq // GROUPS, D], F32,
                           kind="ExternalOutput")

    EXP = mybir.ActivationFunctionType.Exp

    with tile.TileContext(nc) as tc:
        with (
            tc.tile_pool(name="consts", bufs=1) as consts,
            tc.tile_pool(name="wqkv", bufs=3) as wqkv_pool,
            tc.tile_pool(name="wop", bufs=1) as wo_pool,
            tc.tile_pool(name="xt", bufs=5) as xt_pool,
            tc.tile_pool(name="acts", bufs=1) as acts,
            tc.tile_pool(name="ctp", bufs=2) as ct_pool,
            tc.tile_pool(name="epool", bufs=4) as epool,
            tc.tile_pool(name="small", bufs=2) as small,
            tc.tile_pool(name="ysb", bufs=4) as ysb,
            tc.tile_pool(name="psum_mm", bufs=2, space="PSUM") as psum_mm,
            tc.tile_pool(name="psum_s", bufs=2, space="PSUM") as psum_s,
            tc.tile_pool(name="psum_u", bufs=2, space="PSUM") as psum_u,
            tc.tile_pool(name="dram", bufs=1, space="DRAM") as dram,
        ):
            # PE p-state warmup: dummy matmuls spread out by ping-ponging
            # through a DVE copy (two semaphore hops each, ~400ns apart) so
            # the tensor engine never idles long enough to reset its clock
            # ramp while the initial DMAs fill SBUF.
            zt = consts.tile([P, P], BF16, tag="warm")
            nc.gpsimd.memset(zt[:], 0.0)
            wsb = consts.tile([P, 16], F32, tag="warm2")
            pwarm = psum_mm.tile([P, 512], F32, tag="mm", name="pwarm")
            for _ in range(16):
                nc.tensor.matmul(pwarm[:, 0:16], zt[:], zt[:, 0:16],
                                 start=True, stop=True)
                nc.vector.tensor_copy(wsb[:], pwarm[:, 0:16])

            def xpose(dst, x_dram, sb):
                # x rows [sb*512,(sb+1)*512) -> dst[:, dc, :] = slab^T (bf16).
                # One xbar instruction transposes four 128-col blocks into the
                # 3D [128, 4, 512] layout directly (in [512, 4*128] reshaped
                # (512,4,128) then reversed-transposed is exactly d-major).
                for g in range(d_chunks // 4):
                    nc.sync.dma_start_transpose(
                        dst[:, 4 * g:4 * (g + 1), :],
                        x_dram[sb * 512:(sb + 1) * 512,
                               g * 512:(g + 1) * 512])

            # -- DMA order: wk first (first kproj needs it), then x2 slab0
            #    transposes so kproj starts ASAP --
            x2Ts = [xt_pool.tile([P, d_chunks, 512], BF16, tag="xT",
                                 name=f"x2T_{sb}") for sb in range(n_slabs)]
            wk_sb = wqkv_pool.tile([P, d_chunks, JG], BF16, tag="wqkv")
            wv_sb = wqkv_pool.tile([P, d_chunks, JG], BF16, tag="wqkv")
            wq_sb = wqkv_pool.tile([P, d_chunks, JG], BF16, tag="wqkv")
            nc.sync.dma_start(wk_sb[:],
                              wk.rearrange("(o p) j -> p o j", p=P))
            nc.sync.dma_start(wv_sb[:],
                              wv.rearrange("(o p) j -> p o j", p=P))
            xpose(x2Ts[0], x2r, 0)
            bq_sb = consts.tile([P, j_chunks], F32, tag="bq")
            bk_sb = consts.tile([P, j_chunks], F32, tag="bk")
            nc.sync.dma_start(bq_sb[:], bqr[:])
            nc.sync.dma_start(bk_sb[:], bkr[:])
            x1Ts = [xt_pool.tile([P, d_chunks, 512], BF16, tag="xT",
                                 name=f"x1T_{sb}") for sb in range(n_slabs)]
            xpose(x2Ts[1], x2r, 1)
            xpose(x2Ts[2], x2r, 2)
            xpose(x1Ts[0], x1r, 0)
            xpose(x2Ts[3], x2r, 3)
            nc.sync.dma_start(wq_sb[:],
                              wq.rearrange("(o p) j -> p o j", p=P))
            wo_sb = wo_pool.tile([P, j_chunks, D], F32, tag="wo")
            for o in range(j_chunks):
                st = ysb.tile([P, D], F32, tag="y", name=f"wst_{o}")
                nc.sync.dma_start(
                    st[:], wo.rearrange("(o p) n -> p o n", p=P)[:, o, :])
                nc.vector.tensor_copy(r(wo_sb[:, o, :]), st[:])

            # -- persistent activations --
            kT = acts.tile([P, j_chunks, seq], F32, tag="kT")
            qT = acts.tile([P, j_chunks, seq], F32, tag="qT")
            # V'' per head-column-block: cols 0..63 V_h, 64..127 ones
            vpp = acts.tile([P, s_chunks, HPC * P], F32, tag="vpp")

            ones_f32 = consts.tile([P, DH], F32, tag="ones")
            nc.vector.memset(ones_f32[:], 1.0)
            for si in range(s_chunks):
                ones_view = vpp[:, si].rearrange(
                    "p (h q) -> p h q", q=P)[:, :, DH:P]
                # scalar engine is idle before attention; it also rounds f32r
                nc.scalar.copy(
                    r(ones_view),
                    ones_f32[:, None, :].to_broadcast([P, HPC, DH]))

            def project_jmajor(xT_s, w_sb, sb, out, bias):
                # out[:, jc, sb-slab] = w.T @ x^T + bias (j-major)
                for jc in range(j_chunks):
                    pk = psum_mm.tile([P, 512], F32, tag="mm")
                    for dc in range(d_chunks):
                        nc.tensor.matmul(
                            pk[:],
                            w_sb[:, dc, jc * P:(jc + 1) * P],
                            xT_s[:, dc, :],
                            start=(dc == 0), stop=(dc == d_chunks - 1))
                    nc.vector.tensor_scalar_add(
                        r(out[:, jc, sb * 512:(sb + 1) * 512]),
                        pk[:], bias[:, jc:jc + 1])

            def qproj_pieces(sb):
                # the same j-major projection as above, split into 4 closures
                # (~850ns of PE each) so it can be drip-fed between attention
                # units while the scalar engine works through the exps
                state = {}

                def half(jc, lo):
                    def go():
                      with nc.named_scope("qproj"):
                        if lo == 0:
                            state[jc] = psum_mm.tile([P, 512], F32, tag="mm",
                                                     name=f"pq_{sb}_{jc}")
                        pk = state[jc]
                        for dc in range(lo, lo + 4):
                            nc.tensor.matmul(
                                pk[:],
                                wq_sb[:, dc, jc * P:(jc + 1) * P],
                                x1Ts[sb][:, dc, :],
                                start=(dc == 0), stop=(dc == d_chunks - 1))
                        if lo == 4:
                            nc.vector.tensor_scalar_add(
                                r(qT[:, jc, sb * 512:(sb + 1) * 512]),
                                pk[:], bq_sb[:, jc:jc + 1])
                    return go

                return [half(jc, lo) for jc in range(j_chunks)
                        for lo in (0, 4)]

            def project_v(xT_s, sb):
                # V[s-slab, :] = x2_slab @ Wv into the vpp head blocks
                for q in range(4):
                    si = sb * 4 + q
                    pv = psum_u.tile([P, 512], F32, tag="u")
                    for dc in range(d_chunks):
                        nc.tensor.matmul(
                            pv[:, 0:JG],
                            xT_s[:, dc, q * P:(q + 1) * P],
                            wv_sb[:, dc, :],
                            start=(dc == 0), stop=(dc == d_chunks - 1))
                    vv = vpp[:, si].rearrange(
                        "p (h q) -> p h q", q=P)[:, :, 0:DH]
                    nc.vector.tensor_copy(
                        r(vv),
                        pv[:, 0:JG].rearrange("p (h q) -> p h q", q=DH))

            # -- x2 -> K^T, V''; x1 transposes stream behind on the DMA --
            for sb in range(n_slabs):
                with nc.named_scope("kproj"):
                    project_jmajor(x2Ts[sb], wk_sb, sb, kT, bk_sb)
                with nc.named_scope("vproj"):
                    project_v(x2Ts[sb], sb)
                if sb >= 1:
                    # x1T slab sb reuses x2T slab sb-1's pool slot; emit its
                    # DMA only after that slab's readers (kproj/vproj above)
                    xpose(x1Ts[sb], x1r, sb)
            with nc.named_scope("qproj"):
                project_jmajor(x1Ts[0], wq_sb, 0, qT, bq_sb)

            ybounce = dram.tile([seq, D], F32, tag="yin")

            cts = {}
            pus = {}

            def oproj_piece(sc, cT, s8):
                def go():
                  with nc.named_scope("oproj"):
                    yt = ysb.tile([P, D], F32, tag="y",
                                  name=f"yt_{sc}_{s8}")
                    last = sc == n_slabs - 1
                    for nck in range(2):
                        py = psum_mm.tile([P, 512], F32, tag="mm",
                                          name=f"py_{sc}_{s8}_{nck}")
                        for jc in range(j_chunks):
                            nc.tensor.matmul(
                                py[:],
                                mm(cT[:, jc, s8 * P:(s8 + 1) * P]),
                                mm(wo_sb[:, jc, nck * 512:(nck + 1) * 512]),
                                start=(jc == 0), stop=(jc == j_chunks - 1))
                        if last and nck == 1:
                            # the scalar engine is drained of exps at the
                            # very end; splitting the final evictions across
                            # ACT+DVE shortens the tail
                            nc.scalar.copy(
                                yt[:, nck * 512:(nck + 1) * 512], py[:])
                        else:
                            nc.vector.tensor_copy(
                                yt[:, nck * 512:(nck + 1) * 512], py[:])
                    si = sc * 4 + s8
                    if with_collective or sc > 0:
                        nc.sync.dma_start(
                            ybounce[si * P:(si + 1) * P, :], yt[:])
                    else:
                        # timed (no-collective) build: the final DRAM->DRAM
                        # copy stands in for the untimed ReduceScatter, so
                        # write the covered rows straight to the output
                        nc.sync.dma_start(
                            y_out[si * P:(si + 1) * P, :], yt[:])
                return go

            def emit_pv(sc, h, kcp, et):
              with nc.named_scope("attn"):
                jc, po = h // 2, (h % 2) * DH
                if kcp == 0:
                    pus[(sc, h)] = psum_u.tile([P, 512], F32, tag="u",
                                               name=f"pu_{sc}_{h}")
                pu = pus[(sc, h)]
                for dk in range(2):
                    kc = kcp * 2 + dk
                    nc.tensor.matmul(
                        pu[:],
                        mm(vpp[:, kc, h * P:(h + 1) * P]),
                        mm(et[:, dk * 512:(dk + 1) * 512]),
                        start=(kcp == 0 and dk == 0),
                        stop=(kcp == n_kcp - 1 and dk == 1))
                if kcp == n_kcp - 1:
                    cT = cts[sc]
                    rt = small.tile([DH, 512], F32, tag="rt",
                                    name=f"rt_{sc}_{h}")
                    nc.vector.reciprocal(rt[:], pu[DH:P, :])
                    nc.vector.tensor_mul(
                        r(cT[po:po + DH, jc, :]), pu[0:DH, :], rt[:])
                    del pus[(sc, h)]
                    if h == HPC - 1:
                        cT_done = cts.pop(sc)
                        for s8 in range(4):
                            fill.append(oproj_piece(sc, cT_done, s8))

            pend = []
            import collections as _c
            fill = _c.deque()

            def emit_attn_unit(sc, h, kcp):
              with nc.named_scope("attn"):
                if (h, kcp) == (0, 0):
                    cts[sc] = ct_pool.tile([P, j_chunks, 512], F32,
                                           tag="cT", name=f"cT_{sc}")
                jc, po = h // 2, (h % 2) * DH
                ps = psum_s.tile([P, 1024], F32, tag="s",
                                 name=f"ps_{sc}_{h}_{kcp}")
                for dk in range(2):
                    kc = kcp * 2 + dk
                    nc.tensor.matmul(
                        ps[:, dk * 512:(dk + 1) * 512],
                        mm(kT[po:po + DH, jc, kc * P:(kc + 1) * P]),
                        mm(qT[po:po + DH, jc, sc * 512:(sc + 1) * 512]),
                        start=True, stop=True)
                et = epool.tile([P, 1024], F32, tag="e",
                                name=f"et_{sc}_{h}_{kcp}")
                nc.scalar.activation(r(et[:]), ps[:], EXP, scale=0.125)
                pend.append((sc, h, kcp, et))
                if len(pend) > lag:
                    emit_pv(*pend.pop(0))

            # -- attention: 4 chunks of 512 queries. The next chunk's
            #    Q-projection and the previous chunk's out-projection are
            #    drip-fed from the fill queue, one ~850ns piece per unit,
            #    so the PE stays busy while ACT works through the exps --
            for sc in range(n_slabs):
                if sc + 1 < n_slabs:
                    fill.extend(qproj_pieces(sc + 1))
                for h in range(HPC):
                    for kcp in range(n_kcp):
                        emit_attn_unit(sc, h, kcp)
                        if fill:
                            fill.popleft()()
            with nc.named_scope("attn"):
                for args in pend:
                    emit_pv(*args)
                while fill:
                    fill.popleft()()

            # -- sum partials across the 4 cores of this batch --
            # Two half-sized ReduceScatters: the first depends only on the
            # first 1024 rows, so it overlaps the second half's attention.
            if with_collective:
                half = seq // 2                 # 1024 rows per collective
                qr = seq // GROUPS // 2         # 256 rows per rank per half
                for ci in range(2):
                    ysc = dram.tile([qr, D], F32, tag="yout",
                                    name=f"ysc_{ci}")
                    nc.gpsimd.collective_compute(
                        "ReduceScatter",
                        mybir.AluOpType.add,
                        replica_groups=[[0, 1, 2, 3], [4, 5, 6, 7]],
                        ins=[ybounce[ci * half:(ci + 1) * half, :].opt()],
                        outs=[ysc[:].opt()],
                    )
                    nc.sync.dma_start(y_out[ci * qr:(ci + 1) * qr, :], ysc[:])
            # (no-collective build: y_out rows were written directly by
            # emit_oproj's sc==0 DMAs)

    nc.compile()
    return nc


def _get_program(seq=SEQ):
    if seq not in _cached:
        _cached[seq] = _build_program(seq)
    return _cached[seq]


def make_in_maps(x1, x2, Wq, bq, Wk, bk, Wv, bv, Wo, bo):
    """Per-core input dicts for the SPMD program (x and Wqkv host-cast to
    bf16; attention itself stays f32r on-chip)."""
    import ml_dtypes
    bf16 = ml_dtypes.bfloat16
    x1 = np.asarray(x1, np.float32).astype(bf16)
    x2 = np.asarray(x2, np.float32).astype(bf16)
    Wqh = np.asarray(Wq, np.float32).astype(bf16)
    Wkh = np.asarray(Wk, np.float32).astype(bf16)
    Wvh = np.asarray(Wv, np.float32).astype(bf16)
    Wo = np.asarray(Wo, np.float32)
    bq = np.asarray(bq, np.float32)
    bk = np.asarray(bk, np.float32)
    in_maps = []
    for c in range(N_CORES):
        b, g = c // GROUPS, c % GROUPS
        js = slice(g * JG, (g + 1) * JG)
        in_maps.append({
            "x1r": np.ascontiguousarray(x1[b]),
            "x2r": np.ascontiguousarray(x2[b]),
            "wq": np.ascontiguousarray(Wqh[:, js]),
            "wk": np.ascontiguousarray(Wkh[:, js]),
            "wv": np.ascontiguousarray(Wvh[:, js]),
            "wo": np.ascontiguousarray(Wo[js, :]),
            "bqr": np.ascontiguousarray(bq[js].reshape(2, P).T),
            "bkr": np.ascontiguousarray(bk[js].reshape(2, P).T),
        })
    return in_maps


def assemble(results, Wv_bias_fix):
    """results: list of per-core {'y_out': [seq//GROUPS, D]}.

    y_out rows [0:q) = rank's quarter of input rows [0:seq/2);
    rows [q:2q) = rank's quarter of input rows [seq/2:seq)."""
    seq = results[0]["y_out"].shape[0] * GROUPS
    q = seq // GROUPS // 2
    Y = np.empty((B, seq, D), np.float32)
    for c in range(N_CORES):
        b, rr = c // GROUPS, c % GROUPS
        yo = results[c]["y_out"]
        Y[b, rr * q:(rr + 1) * q, :] = yo[:q]
        Y[b, seq // 2 + rr * q:seq // 2 + (rr + 1) * q, :] = yo[q:]
    Y += Wv_bias_fix
    return Y


def kernel(x1, x2, Wq, bq, Wk, bk, Wv, bv, Wo, bo):
    from concourse.bass_utils import run_bass_kernel_spmd

    Wo = np.asarray(Wo, np.float32)
    bv = np.asarray(bv, np.float32)
    bo = np.asarray(bo, np.float32)

    nc = _get_program(SEQ)
    in_maps = make_in_maps(x1, x2, Wq, bq, Wk, bk, Wv, bv, Wo, bo)
    res = run_bass_kernel_spmd(nc, in_maps, core_ids=list(range(N_CORES)))
    fix = (bv @ Wo + bo).astype(np.float32)
    return assemble(res.results, fix)


# revision 25
# speedup vs baseline: 1.2656x; 1.0184x over previous
"""Multi-head cross-attention on 8 Trainium2 NeuronCores.

Sharding: data-parallel over batch (2) x tensor-parallel over heads (4 groups
of 4 heads). Core c handles batch c//4, head-group c%4 (a 256-wide slice of
the QKV projection space). Each core computes a partial output-projection
Y_partial = ctx_c @ Wo_c; a ReduceScatter(add) over each batch's 4 cores
leaves each core with a 512-row shard of the summed output, which the host
concatenates.

On-core dataflow:
  - x1/x2 arrive as bf16 (host-cast); x^T is produced by the DMA xbar
    (dma_start_transpose, 16x128 tiles) straight from DRAM -- the PE does no
    transposes at all. QKV projections run bf16 x bf16 into f32 PSUM.
  - Q^T/K^T = W.T @ x^T come out j-major, V = x @ Wv comes out s-major --
    exactly the operand layouts the attention matmuls need.
  - attention runs in f32r at full PE rate, tiled as (512-query chunk sc,
    head h, key-chunk pair): scores for two 128-key chunks land in one
    [128,1024] PSUM tile and are exp'd in a single scalar-engine op (no max
    subtraction: logits ~ N(0,1)). V carries 64 ones-columns so the softmax
    denominator accumulates in PSUM partitions 64..127 of the same PV
    matmul chain; one reciprocal+multiply normalizes into cT.
  - the PV stream lags the exp stream by a few units, and the next chunk's
    Q-projection plus the previous chunk's out-projection are emitted inside
    the attention stream so the PE never starves while the scalar engine
    works through the exps.
  - bq/bk are applied on-device (per-partition bias in j-major layout).
    bv/bo commute through softmax/out-projection exactly (softmax rows sum
    to 1), so the host adds bv @ Wo + bo to the final output.
  - a zero-matmul warms the PE p-state ramp during the initial DMA fill.
"""

import numpy as np

B, SEQ, D, H, DH = 2, 2048, 1024, 16, 64
N_CORES = 8
GROUPS = 4            # head-groups per batch (cores per batch)
JG = D // GROUPS      # 256 projection dims per core
HPC = H // GROUPS     # 4 heads per core
P = 128

_cached = {}


def _build_program(seq=SEQ, with_collective=True, lag=3):
    import concourse.tile as tile
    from concourse import bacc, mybir

    F32 = mybir.dt.float32
    BF16 = mybir.dt.bfloat16
    F32R = mybir.dt.float32r

    def r(x):
        return x.bitcast(F32R)

    mm = r  # matmul operands are f32r views of f32 tiles

    d_chunks = D // P            # 8
    j_chunks = JG // P           # 2
    n_slabs = seq // 512         # 4 (512-row x blocks and 512-query chunks)
    s_chunks = seq // P          # 16 (128-key chunks)
    n_kcp = s_chunks // 2        # 8 key-chunk pairs per (sc, h)

    nc = bacc.Bacc("TRN2", target_bir_lowering=False, debug=False,
                   num_devices=N_CORES)

    x1r = nc.dram_tensor("x1r", [seq, D], BF16, kind="ExternalInput")
    x2r = nc.dram_tensor("x2r", [seq, D], BF16, kind="ExternalInput")
    wq = nc.dram_tensor("wq", [D, JG], BF16, kind="ExternalInput")
    wk = nc.dram_tensor("wk", [D, JG], BF16, kind="ExternalInput")
    wv = nc.dram_tensor("wv", [D, JG], BF16, kind="ExternalInput")
    wo = nc.dram_tensor("wo", [JG, D], F32, kind="ExternalInput")
    bqr = nc.dram_tensor("bqr", [P, j_chunks], F32, kind="ExternalInput")
    bkr = nc.dram_tensor("bkr", [P, j_chunks], F32, kind="ExternalInput")
    y_out = nc.dram_tensor("y_out", [seq // GROUPS, D], F32,
                           kind="ExternalOutput")

    EXP = mybir.ActivationFunctionType.Exp

    with tile.TileContext(nc) as tc:
        with (
            tc.tile_pool(name="consts", bufs=1) as consts,
            tc.tile_pool(name="wqkv", bufs=3) as wqkv_pool,
            tc.tile_pool(name="wop", bufs=1) as wo_pool,
            tc.tile_pool(name="xt", bufs=5) as xt_pool,
            tc.tile_pool(name="acts", bufs=1) as acts,
            tc.tile_pool(name="ctp", bufs=2) as ct_pool,
            tc.tile_pool(name="epool", bufs=4) as epool,
            tc.tile_pool(name="small", bufs=2) as small,
            tc.tile_pool(name="ysb", bufs=4) as ysb,
            tc.tile_pool(name="psum_mm", bufs=1, space="PSUM") as psum_mm,
            tc.tile_pool(name="psum_q", bufs=1, space="PSUM") as psum_q,
            tc.tile_pool(name="psum_s", bufs=2, space="PSUM") as psum_s,
            tc.tile_pool(name="psum_u", bufs=2, space="PSUM") as psum_u,
            tc.tile_pool(name="dram", bufs=1, space="DRAM") as dram,
        ):
            # PE p-state warmup: dummy matmuls spread out by ping-ponging
            # through a DVE copy (two semaphore hops each, ~400ns apart) so
            # the tensor engine never idles long enough to reset its clock
            # ramp while the initial DMAs fill SBUF.
            zt = consts.tile([P, P], BF16, tag="warm")
            nc.gpsimd.memset(zt[:], 0.0)
            wsb = consts.tile([P, 16], F32, tag="warm2")
            pwarm = psum_mm.tile([P, 512], F32, tag="mm", name="pwarm")
            for _ in range(12):
                nc.tensor.matmul(pwarm[:, 0:16], zt[:], zt[:, 0:16],
                                 start=True, stop=True)
                nc.vector.tensor_copy(wsb[:], pwarm[:, 0:16])

            def xpose(dst, x_dram, sb):
                # x rows [sb*512,(sb+1)*512) -> dst[:, dc, :] = slab^T (bf16).
                # One xbar instruction transposes four 128-col blocks into the
                # 3D [128, 4, 512] layout directly (in [512, 4*128] reshaped
                # (512,4,128) then reversed-transposed is exactly d-major).
                for g in range(d_chunks // 4):
                    nc.sync.dma_start_transpose(
                        dst[:, 4 * g:4 * (g + 1), :],
                        x_dram[sb * 512:(sb + 1) * 512,
                               g * 512:(g + 1) * 512])

            # -- DMA order: wk first (first kproj needs it), then x2 slab0
            #    transposes so kproj starts ASAP --
            x2Ts = [xt_pool.tile([P, d_chunks, 512], BF16, tag="xT",
                                 name=f"x2T_{sb}") for sb in range(n_slabs)]
            wk_sb = wqkv_pool.tile([P, d_chunks, JG], BF16, tag="wqkv")
            wv_sb = wqkv_pool.tile([P, d_chunks, JG], BF16, tag="wqkv")
            wq_sb = wqkv_pool.tile([P, d_chunks, JG], BF16, tag="wqkv")
            nc.sync.dma_start(wk_sb[:],
                              wk.rearrange("(o p) j -> p o j", p=P))
            nc.sync.dma_start(wv_sb[:],
                              wv.rearrange("(o p) j -> p o j", p=P))
            bq_sb = consts.tile([P, j_chunks], F32, tag="bq")
            bk_sb = consts.tile([P, j_chunks], F32, tag="bk")
            nc.sync.dma_start(bq_sb[:], bqr[:])
            nc.sync.dma_start(bk_sb[:], bkr[:])
            x1Ts = [xt_pool.tile([P, d_chunks, 512], BF16, tag="xT",
                                 name=f"x1T_{sb}") for sb in range(n_slabs)]
            for sb in range(n_slabs):
                xpose(x2Ts[sb], x2r, sb)
            xpose(x1Ts[0], x1r, 0)
            nc.sync.dma_start(wq_sb[:],
                              wq.rearrange("(o p) j -> p o j", p=P))
            wo_sb = wo_pool.tile([P, j_chunks, D], F32, tag="wo")
            for o in range(j_chunks):
                st = ysb.tile([P, D], F32, tag="y", name=f"wst_{o}")
                nc.sync.dma_start(
                    st[:], wo.rearrange("(o p) n -> p o n", p=P)[:, o, :])
                nc.vector.tensor_copy(r(wo_sb[:, o, :]), st[:])

            # -- persistent activations --
            kT = acts.tile([P, j_chunks, seq], F32, tag="kT")
            qT = acts.tile([P, j_chunks, seq], F32, tag="qT")
            # V'' per head-column-block: cols 0..63 V_h, 64..127 ones
            vpp = acts.tile([P, s_chunks, HPC * P], F32, tag="vpp")

            ones_f32 = consts.tile([P, DH], F32, tag="ones")
            nc.vector.memset(ones_f32[:], 1.0)
            for si in range(s_chunks):
                ones_view = vpp[:, si].rearrange(
                    "p (h q) -> p h q", q=P)[:, :, DH:P]
                # scalar engine is idle before attention; it also rounds f32r
                nc.scalar.copy(
                    r(ones_view),
                    ones_f32[:, None, :].to_broadcast([P, HPC, DH]))

            def project_jmajor(xT_s, w_sb, sb, out, bias):
                # out[:, jc, sb-slab] = w.T @ x^T + bias (j-major); the two
                # jc chains use separate single-buffer pools so they overlap
                for jc in range(j_chunks):
                    pool = psum_q if jc == 0 else psum_mm
                    pk = pool.tile([P, 512], F32,
                                   tag=("q" if jc == 0 else "mm"),
                                   name=f"pk_{w_sb.name}_{sb}_{jc}")
                    for dc in range(d_chunks):
                        nc.tensor.matmul(
                            pk[:],
                            w_sb[:, dc, jc * P:(jc + 1) * P],
                            xT_s[:, dc, :],
                            start=(dc == 0), stop=(dc == d_chunks - 1))
                    nc.vector.tensor_scalar_add(
                        r(out[:, jc, sb * 512:(sb + 1) * 512]),
                        pk[:], bias[:, jc:jc + 1])

            def jproj_pieces(w_sb, xT, sb, out, bias, scope, step=2):
                # j-major projection split into ~425ns closures drip-fed
                # between attention units; the dedicated single-buffer
                # psum_q pool holds the open accumulation chain (the two jc
                # chains run back to back, never concurrently)
                state = {}

                def piece(jc, lo):
                    def go():
                      with nc.named_scope(scope):
                        if lo == 0:
                            state[jc] = psum_q.tile(
                                [P, 512], F32, tag="q",
                                name=f"pj_{scope}_{sb}_{jc}")
                        pk = state[jc]
                        for dc in range(lo, lo + step):
                            nc.tensor.matmul(
                                pk[:],
                                w_sb[:, dc, jc * P:(jc + 1) * P],
                                xT[:, dc, :],
                                start=(dc == 0), stop=(dc == d_chunks - 1))
                        if lo + step == d_chunks:
                            nc.vector.tensor_scalar_add(
                                r(out[:, jc, sb * 512:(sb + 1) * 512]),
                                pk[:], bias[:, jc:jc + 1])
                    return go

                return [piece(jc, lo) for jc in range(j_chunks)
                        for lo in range(0, d_chunks, step)]

            def qproj_pieces(sb):
                return jproj_pieces(wq_sb, x1Ts[sb], sb, qT, bq_sb,
                                    "qproj", step=2)

            def vproj_piece(sb, q, pool=None, tag="u"):
                # fill-time pieces must NOT use psum_u: its round-robin slot
                # may hold a live PV accumulator mid-attention
                def go():
                  with nc.named_scope("vproj"):
                    si = sb * 4 + q
                    pv = (pool or psum_u).tile([P, 512], F32, tag=tag,
                                               name=f"pv_{si}")
                    for dc in range(d_chunks):
                        nc.tensor.matmul(
                            pv[:, 0:JG],
                            x2Ts[sb][:, dc, q * P:(q + 1) * P],
                            wv_sb[:, dc, :],
                            start=(dc == 0), stop=(dc == d_chunks - 1))
                    vv = vpp[:, si].rearrange(
                        "p (h q) -> p h q", q=P)[:, :, 0:DH]
                    nc.vector.tensor_copy(
                        r(vv),
                        pv[:, 0:JG].rearrange("p (h q) -> p h q", q=DH))
                return go

            def project_v(xT_s, sb):
                # V[s-slab, :] = x2_slab @ Wv into the vpp head blocks
                for q in range(4):
                    vproj_piece(sb, q)()

            # -- x2 -> K^T, V''; x1 transposes stream behind on the DMA.
            #    qproj0 runs before the last K slab so attention can start
            #    immediately after; vproj slab3 is deferred into the fill
            #    queue (its vpp rows are first read several units in) --
            for sb in range(n_slabs - 1):
                with nc.named_scope("kproj"):
                    project_jmajor(x2Ts[sb], wk_sb, sb, kT, bk_sb)
                with nc.named_scope("vproj"):
                    project_v(x2Ts[sb], sb)
                # x1T slab sb+1 reuses x2T slab sb's pool slot; emit its
                # DMA only after that slab's readers (kproj/vproj above)
                xpose(x1Ts[sb + 1], x1r, sb + 1)
            with nc.named_scope("qproj"):
                project_jmajor(x1Ts[0], wq_sb, 0, qT, bq_sb)

            ybounce = dram.tile([seq, D], F32, tag="yin")

            cts = {}
            pus = {}

            yts = {}

            def oproj_piece(sc, cT, s8, nck):
                def go():
                  with nc.named_scope("oproj"):
                    if nck == 0:
                        yts[(sc, s8)] = ysb.tile([P, D], F32, tag="y",
                                                 name=f"yt_{sc}_{s8}")
                    yt = yts[(sc, s8)]
                    last = sc == n_slabs - 1
                    py = psum_mm.tile([P, 512], F32, tag="mm",
                                      name=f"py_{sc}_{s8}_{nck}")
                    for jc in range(j_chunks):
                        nc.tensor.matmul(
                            py[:],
                            mm(cT[:, jc, s8 * P:(s8 + 1) * P]),
                            mm(wo_sb[:, jc, nck * 512:(nck + 1) * 512]),
                            start=(jc == 0), stop=(jc == j_chunks - 1))
                    if last and nck == 1:
                        # the scalar engine is drained of exps at the very
                        # end; splitting the final evictions across ACT+DVE
                        # shortens the tail
                        nc.scalar.copy(
                            yt[:, nck * 512:(nck + 1) * 512], py[:])
                    else:
                        nc.vector.tensor_copy(
                            yt[:, nck * 512:(nck + 1) * 512], py[:])
                    if nck == 1:
                        del yts[(sc, s8)]
                        si = sc * 4 + s8
                        if with_collective or sc > 0:
                            nc.sync.dma_start(
                                ybounce[si * P:(si + 1) * P, :], yt[:])
                        else:
                            # timed (no-collective) build: the final
                            # DRAM->DRAM copy stands in for the untimed
                            # ReduceScatter, so write the covered rows
                            # straight to the output
                            nc.sync.dma_start(
                                y_out[si * P:(si + 1) * P, :], yt[:])
                return go

            def emit_pv(sc, h, kcp, et):
              with nc.named_scope("attn"):
                jc, po = h // 2, (h % 2) * DH
                if kcp == 0:
                    pus[(sc, h)] = psum_u.tile([P, 512], F32, tag="u",
                                               name=f"pu_{sc}_{h}")
                pu = pus[(sc, h)]
                for dk in range(2):
                    kc = kcp * 2 + dk
                    nc.tensor.matmul(
                        pu[:],
                        mm(vpp[:, kc, h * P:(h + 1) * P]),
                        mm(et[:, dk * 512:(dk + 1) * 512]),
                        start=(kcp == 0 and dk == 0),
                        stop=(kcp == n_kcp - 1 and dk == 1))
                if kcp == n_kcp - 1:
                    cT = cts[sc]
                    rt = small.tile([DH, 512], F32, tag="rt",
                                    name=f"rt_{sc}_{h}")
                    nc.vector.reciprocal(rt[:], pu[DH:P, :])
                    nc.vector.tensor_mul(
                        r(cT[po:po + DH, jc, :]), pu[0:DH, :], rt[:])
                    del pus[(sc, h)]
                    if h == HPC - 1:
                        cT_done = cts.pop(sc)
                        for s8 in range(4):
                            for nck in range(2):
                                fill.append(
                                    oproj_piece(sc, cT_done, s8, nck))

            pend = []
            import collections as _c
            fill = _c.deque()

            def emit_attn_unit(sc, h, kcp):
              with nc.named_scope("attn"):
                if (h, kcp) == (0, 0):
                    cts[sc] = ct_pool.tile([P, j_chunks, 512], F32,
                                           tag="cT", name=f"cT_{sc}")
                jc, po = h // 2, (h % 2) * DH
                ps = psum_s.tile([P, 1024], F32, tag="s",
                                 name=f"ps_{sc}_{h}_{kcp}")
                for dk in range(2):
                    kc = kcp * 2 + dk
                    nc.tensor.matmul(
                        ps[:, dk * 512:(dk + 1) * 512],
                        mm(kT[po:po + DH, jc, kc * P:(kc + 1) * P]),
                        mm(qT[po:po + DH, jc, sc * 512:(sc + 1) * 512]),
                        start=True, stop=True)
                et = epool.tile([P, 1024], F32, tag="e",
                                name=f"et_{sc}_{h}_{kcp}")
                nc.scalar.activation(r(et[:]), ps[:], EXP, scale=0.125)
                pend.append((sc, h, kcp, et))
                if len(pend) > lag:
                    emit_pv(*pend.pop(0))

            # -- attention: 4 chunks of 512 queries. The next chunk's
            #    Q-projection and the previous chunk's out-projection are
            #    drip-fed from the fill queue, one ~850ns piece per unit,
            #    so the PE stays busy while ACT works through the exps --
            # slab3's K and V projections are drip-fed at the start of
            # attention (kT slab3 is first read at unit 6, vpp rows 12-15
            # at unit 6+lag), so the attention stream starts ~5us earlier
            kp3 = jproj_pieces(wk_sb, x2Ts[3], 3, kT, bk_sb,
                               "kproj", step=4)
            vp3 = [vproj_piece(3, q, pool=psum_mm, tag="mm")
                   for q in range(4)]
            for a, b in zip(kp3, vp3):
                fill.append(a)
                fill.append(b)
            for sc in range(n_slabs):
                if sc + 1 < n_slabs:
                    fill.extend(qproj_pieces(sc + 1))
                for h in range(HPC):
                    for kcp in range(n_kcp):
                        emit_attn_unit(sc, h, kcp)
                        u = h * n_kcp + kcp
                        if sc == 0 and u < 8 and u % 2 == 0:
                            # double-pop: slab3's deferred K/V projections
                            # must land before units 6..10 consume them
                            for _ in range(min(2, len(fill))):
                                fill.popleft()()
                        elif fill and (
                                u % 2 == 0 if sc < n_slabs - 1
                                else (u < 8 or h == HPC - 1)):
                            fill.popleft()()
            with nc.named_scope("attn"):
                for args in pend:
                    emit_pv(*args)
                    if fill:
                        fill.popleft()()
                while fill:
                    fill.popleft()()

            # -- sum partials across the 4 cores of this batch --
            # Two half-sized ReduceScatters: the first depends only on the
            # first 1024 rows, so it overlaps the second half's attention.
            if with_collective:
                half = seq // 2                 # 1024 rows per collective
                qr = seq // GROUPS // 2         # 256 rows per rank per half
                for ci in range(2):
                    ysc = dram.tile([qr, D], F32, tag="yout",
                                    name=f"ysc_{ci}")
                    nc.gpsimd.collective_compute(
                        "ReduceScatter",
                        mybir.AluOpType.add,
                        replica_groups=[[0, 1, 2, 3], [4, 5, 6, 7]],
                        ins=[ybounce[ci * half:(ci + 1) * half, :].opt()],
                        outs=[ysc[:].opt()],
                    )
                    nc.sync.dma_start(y_out[ci * qr:(ci + 1) * qr, :], ysc[:])
            # (no-collective build: y_out rows were written directly by
            # emit_oproj's sc==0 DMAs)

    nc.compile()
    return nc


def _get_program(seq=SEQ):
    if seq not in _cached:
        _cached[seq] = _build_program(seq)
    return _cached[seq]


def make_in_maps(x1, x2, Wq, bq, Wk, bk, Wv, bv, Wo, bo):
    """Per-core input dicts for the SPMD program (x and Wqkv host-cast to
    bf16; attention itself stays f32r on-chip)."""
    import ml_dtypes
    bf16 = ml_dtypes.bfloat16
    x1 = np.asarray(x1, np.float32).astype(bf16)
    x2 = np.asarray(x2, np.float32).astype(bf16)
    Wqh = np.asarray(Wq, np.float32).astype(bf16)
    Wkh = np.asarray(Wk, np.float32).astype(bf16)
    Wvh = np.asarray(Wv, np.float32).astype(bf16)
    Wo = np.asarray(Wo, np.float32)
    bq = np.asarray(bq, np.float32)
    bk = np.asarray(bk, np.float32)
    in_maps = []
    for c in range(N_CORES):
        b, g = c // GROUPS, c % GROUPS
        js = slice(g * JG, (g + 1) * JG)
        in_maps.append({
            "x1r": np.ascontiguousarray(x1[b]),
            "x2r": np.ascontiguousarray(x2[b]),
            "wq": np.ascontiguousarray(Wqh[:, js]),
            "wk": np.ascontiguousarray(Wkh[:, js]),
            "wv": np.ascontiguousarray(Wvh[:, js]),
            "wo": np.ascontiguousarray(Wo[js, :]),
            "bqr": np.ascontiguousarray(bq[js].reshape(2, P).T),
            "bkr": np.ascontiguousarray(bk[js].reshape(2, P).T),
        })
    return in_maps


def assemble(results, Wv_bias_fix):
    """results: list of per-core {'y_out': [seq//GROUPS, D]}.

    y_out rows [0:q) = rank's quarter of input rows [0:seq/2);
    rows [q:2q) = rank's quarter of input rows [seq/2:seq)."""
    seq = results[0]["y_out"].shape[0] * GROUPS
    q = seq // GROUPS // 2
    Y = np.empty((B, seq, D), np.float32)
    for c in range(N_CORES):
        b, rr = c // GROUPS, c % GROUPS
        yo = results[c]["y_out"]
        Y[b, rr * q:(rr + 1) * q, :] = yo[:q]
        Y[b, seq // 2 + rr * q:seq // 2 + (rr + 1) * q, :] = yo[q:]
    Y += Wv_bias_fix
    return Y


def kernel(x1, x2, Wq, bq, Wk, bk, Wv, bv, Wo, bo):
    from concourse.bass_utils import run_bass_kernel_spmd

    Wo = np.asarray(Wo, np.float32)
    bv = np.asarray(bv, np.float32)
    bo = np.asarray(bo, np.float32)

    nc = _get_program(SEQ)
    in_maps = make_in_maps(x1, x2, Wq, bq, Wk, bk, Wv, bv, Wo, bo)
    res = run_bass_kernel_spmd(nc, in_maps, core_ids=list(range(N_CORES)))
    fix = (bv @ Wo + bo).astype(np.float32)
    return assemble(res.results, fix)


# revision 26
# speedup vs baseline: 1.2890x; 1.0184x over previous
"""Multi-head cross-attention on 8 Trainium2 NeuronCores.

Sharding: data-parallel over batch (2) x tensor-parallel over heads (4 groups
of 4 heads). Core c handles batch c//4, head-group c%4 (a 256-wide slice of
the QKV projection space). Each core computes a partial output-projection
Y_partial = ctx_c @ Wo_c; a ReduceScatter(add) over each batch's 4 cores
leaves each core with a 512-row shard of the summed output, which the host
concatenates.

On-core dataflow:
  - x1/x2 arrive as bf16 (host-cast); x^T is produced by the DMA xbar
    (dma_start_transpose, 16x128 tiles) straight from DRAM -- the PE does no
    transposes at all. QKV projections run bf16 x bf16 into f32 PSUM.
  - Q^T/K^T = W.T @ x^T come out j-major, V = x @ Wv comes out s-major --
    exactly the operand layouts the attention matmuls need.
  - attention runs in f32r at full PE rate, tiled as (512-query chunk sc,
    head h, key-chunk pair): scores for two 128-key chunks land in one
    [128,1024] PSUM tile and are exp'd in a single scalar-engine op (no max
    subtraction: logits ~ N(0,1)). V carries 64 ones-columns so the softmax
    denominator accumulates in PSUM partitions 64..127 of the same PV
    matmul chain; one reciprocal+multiply normalizes into cT.
  - the PV stream lags the exp stream by a few units, and the next chunk's
    Q-projection plus the previous chunk's out-projection are emitted inside
    the attention stream so the PE never starves while the scalar engine
    works through the exps.
  - bq/bk are applied on-device (per-partition bias in j-major layout).
    bv/bo commute through softmax/out-projection exactly (softmax rows sum
    to 1), so the host adds bv @ Wo + bo to the final output.
  - a zero-matmul warms the PE p-state ramp during the initial DMA fill.
"""

import numpy as np

B, SEQ, D, H, DH = 2, 2048, 1024, 16, 64
N_CORES = 8
GROUPS = 4            # head-groups per batch (cores per batch)
JG = D // GROUPS      # 256 projection dims per core
HPC = H // GROUPS     # 4 heads per core
P = 128

_cached = {}


def _build_program(seq=SEQ, with_collective=True, lag=3):
    import concourse.tile as tile
    from concourse import bacc, mybir

    F32 = mybir.dt.float32
    BF16 = mybir.dt.bfloat16
    F32R = mybir.dt.float32r

    def r(x):
        return x.bitcast(F32R)

    mm = r  # matmul operands are f32r views of f32 tiles

    d_chunks = D // P            # 8
    j_chunks = JG // P           # 2
    n_slabs = seq // 512         # 4 (512-row x blocks and 512-query chunks)
    s_chunks = seq // P          # 16 (128-key chunks)
    n_kcp = s_chunks // 2        # 8 key-chunk pairs per (sc, h)

    nc = bacc.Bacc("TRN2", target_bir_lowering=False, debug=False,
                   num_devices=N_CORES)

    x1r = nc.dram_tensor("x1r", [seq, D], BF16, kind="ExternalInput")
    x2r = nc.dram_tensor("x2r", [seq, D], BF16, kind="ExternalInput")
    wq = nc.dram_tensor("wq", [D, JG], BF16, kind="ExternalInput")
    wk = nc.dram_tensor("wk", [D, JG], BF16, kind="ExternalInput")
    wv = nc.dram_tensor("wv", [D, JG], BF16, kind="ExternalInput")
    wo = nc.dram_tensor("wo", [JG, D], F32, kind="ExternalInput")
    bqr = nc.dram_tensor("bqr", [P, j_chunks], F32, kind="ExternalInput")
    bkr = nc.dram_tensor("bkr", [P, j_chunks], F32, kind="ExternalInput")
    y_out = nc.dram_tensor("y_out", [seq // GROUPS, D], F32,
                           kind="ExternalOutput")

    EXP = mybir.ActivationFunctionType.Exp

    with tile.TileContext(nc) as tc:
        with (
            tc.tile_pool(name="consts", bufs=1) as consts,
            tc.tile_pool(name="wqkv", bufs=3) as wqkv_pool,
            tc.tile_pool(name="wop", bufs=1) as wo_pool,
            tc.tile_pool(name="xt", bufs=5) as xt_pool,
            tc.tile_pool(name="acts", bufs=1) as acts,
            tc.tile_pool(name="ctp", bufs=2) as ct_pool,
            tc.tile_pool(name="epool", bufs=4) as epool,
            tc.tile_pool(name="small", bufs=2) as small,
            tc.tile_pool(name="ysb", bufs=4) as ysb,
            tc.tile_pool(name="psum_mm", bufs=1, space="PSUM") as psum_mm,
            tc.tile_pool(name="psum_q", bufs=1, space="PSUM") as psum_q,
            tc.tile_pool(name="psum_s", bufs=2, space="PSUM") as psum_s,
            tc.tile_pool(name="psum_u", bufs=2, space="PSUM") as psum_u,
            tc.tile_pool(name="dram", bufs=1, space="DRAM") as dram,
        ):
            # PE p-state warmup: dummy matmuls spread out by ping-ponging
            # through a DVE copy (two semaphore hops each, ~400ns apart) so
            # the tensor engine never idles long enough to reset its clock
            # ramp while the initial DMAs fill SBUF.
            zt = consts.tile([P, P], BF16, tag="warm")
            nc.gpsimd.memset(zt[:], 0.0)
            wsb = consts.tile([P, 16], F32, tag="warm2")
            pwarm = psum_mm.tile([P, 512], F32, tag="mm", name="pwarm")
            for _ in range(18):
                nc.tensor.matmul(pwarm[:, 0:16], zt[:], zt[:, 0:16],
                                 start=True, stop=True)
                nc.vector.tensor_copy(wsb[:], pwarm[:, 0:16])

            def xpose2(dst, x_dram, sb):
                # finer (2-block) pieces: lower first-chunk latency
                for g in range(d_chunks // 2):
                    nc.sync.dma_start_transpose(
                        dst[:, 2 * g:2 * (g + 1), :],
                        x_dram[sb * 512:(sb + 1) * 512,
                               g * 256:(g + 1) * 256])

            def xpose(dst, x_dram, sb):
                # x rows [sb*512,(sb+1)*512) -> dst[:, dc, :] = slab^T (bf16).
                # One xbar instruction transposes four 128-col blocks into the
                # 3D [128, 4, 512] layout directly (in [512, 4*128] reshaped
                # (512,4,128) then reversed-transposed is exactly d-major).
                for g in range(d_chunks // 4):
                    nc.sync.dma_start_transpose(
                        dst[:, 4 * g:4 * (g + 1), :],
                        x_dram[sb * 512:(sb + 1) * 512,
                               g * 512:(g + 1) * 512])

            # -- DMA order: wk first (first kproj needs it), then x2 slab0
            #    transposes so kproj starts ASAP --
            x2Ts = [xt_pool.tile([P, d_chunks, 512], BF16, tag="xT",
                                 name=f"x2T_{sb}") for sb in range(n_slabs)]
            wk_sb = wqkv_pool.tile([P, d_chunks, JG], BF16, tag="wqkv")
            wv_sb = wqkv_pool.tile([P, d_chunks, JG], BF16, tag="wqkv")
            wq_sb = wqkv_pool.tile([P, d_chunks, JG], BF16, tag="wqkv")
            nc.sync.dma_start(wk_sb[:],
                              wk.rearrange("(o p) j -> p o j", p=P))
            nc.sync.dma_start(wv_sb[:],
                              wv.rearrange("(o p) j -> p o j", p=P))
            bq_sb = consts.tile([P, j_chunks], F32, tag="bq")
            bk_sb = consts.tile([P, j_chunks], F32, tag="bk")
            nc.sync.dma_start(bq_sb[:], bqr[:])
            nc.sync.dma_start(bk_sb[:], bkr[:])
            x1Ts = [xt_pool.tile([P, d_chunks, 512], BF16, tag="xT",
                                 name=f"x1T_{sb}") for sb in range(n_slabs)]
            xpose2(x2Ts[0], x2r, 0)
            for sb in range(1, n_slabs):
                xpose(x2Ts[sb], x2r, sb)
            xpose(x1Ts[0], x1r, 0)
            nc.sync.dma_start(wq_sb[:],
                              wq.rearrange("(o p) j -> p o j", p=P))
            wo_sb = wo_pool.tile([P, j_chunks, D], F32, tag="wo")
            for o in range(j_chunks):
                st = ysb.tile([P, D], F32, tag="y", name=f"wst_{o}")
                nc.sync.dma_start(
                    st[:], wo.rearrange("(o p) n -> p o n", p=P)[:, o, :])
                nc.vector.tensor_copy(r(wo_sb[:, o, :]), st[:])

            # -- persistent activations --
            kT = acts.tile([P, j_chunks, seq], F32, tag="kT")
            qT = acts.tile([P, j_chunks, seq], F32, tag="qT")
            # V'' per head-column-block: cols 0..63 V_h, 64..127 ones
            vpp = acts.tile([P, s_chunks, HPC * P], F32, tag="vpp")

            ones_f32 = consts.tile([P, DH], F32, tag="ones")
            nc.vector.memset(ones_f32[:], 1.0)
            for si in range(s_chunks):
                ones_view = vpp[:, si].rearrange(
                    "p (h q) -> p h q", q=P)[:, :, DH:P]
                # scalar engine is idle before attention; it also rounds f32r
                nc.scalar.copy(
                    r(ones_view),
                    ones_f32[:, None, :].to_broadcast([P, HPC, DH]))

            def project_jmajor(xT_s, w_sb, sb, out, bias, use_act=False):
                # out[:, jc, sb-slab] = w.T @ x^T + bias (j-major); the two
                # jc chains use separate single-buffer pools so they overlap
                for jc in range(j_chunks):
                    pool = psum_q if jc == 0 else psum_mm
                    pk = pool.tile([P, 512], F32,
                                   tag=("q" if jc == 0 else "mm"),
                                   name=f"pk_{w_sb.name}_{sb}_{jc}")
                    for dc in range(d_chunks):
                        nc.tensor.matmul(
                            pk[:],
                            w_sb[:, dc, jc * P:(jc + 1) * P],
                            xT_s[:, dc, :],
                            start=(dc == 0), stop=(dc == d_chunks - 1))
                    if use_act:
                        nc.scalar.add(
                            r(out[:, jc, sb * 512:(sb + 1) * 512]),
                            pk[:], bias[:, jc:jc + 1])
                    else:
                        nc.vector.tensor_scalar_add(
                            r(out[:, jc, sb * 512:(sb + 1) * 512]),
                            pk[:], bias[:, jc:jc + 1])

            def jproj_pieces(w_sb, xT, sb, out, bias, scope, step=2):
                # j-major projection split into ~425ns closures drip-fed
                # between attention units; the dedicated single-buffer
                # psum_q pool holds the open accumulation chain (the two jc
                # chains run back to back, never concurrently)
                state = {}

                def piece(jc, lo):
                    def go():
                      with nc.named_scope(scope):
                        if lo == 0:
                            state[jc] = psum_q.tile(
                                [P, 512], F32, tag="q",
                                name=f"pj_{scope}_{sb}_{jc}")
                        pk = state[jc]
                        for dc in range(lo, lo + step):
                            nc.tensor.matmul(
                                pk[:],
                                w_sb[:, dc, jc * P:(jc + 1) * P],
                                xT[:, dc, :],
                                start=(dc == 0), stop=(dc == d_chunks - 1))
                        if lo + step == d_chunks:
                            nc.vector.tensor_scalar_add(
                                r(out[:, jc, sb * 512:(sb + 1) * 512]),
                                pk[:], bias[:, jc:jc + 1])
                    return go

                return [piece(jc, lo) for jc in range(j_chunks)
                        for lo in range(0, d_chunks, step)]

            def qproj_pieces(sb):
                return jproj_pieces(wq_sb, x1Ts[sb], sb, qT, bq_sb,
                                    "qproj", step=2)

            def vproj_piece(sb, q, pool=None, tag="u"):
                # fill-time pieces must NOT use psum_u: its round-robin slot
                # may hold a live PV accumulator mid-attention
                def go():
                  with nc.named_scope("vproj"):
                    si = sb * 4 + q
                    pv = (pool or psum_u).tile([P, 512], F32, tag=tag,
                                               name=f"pv_{si}")
                    for dc in range(d_chunks):
                        nc.tensor.matmul(
                            pv[:, 0:JG],
                            x2Ts[sb][:, dc, q * P:(q + 1) * P],
                            wv_sb[:, dc, :],
                            start=(dc == 0), stop=(dc == d_chunks - 1))
                    vv = vpp[:, si].rearrange(
                        "p (h q) -> p h q", q=P)[:, :, 0:DH]
                    nc.vector.tensor_copy(
                        r(vv),
                        pv[:, 0:JG].rearrange("p (h q) -> p h q", q=DH))
                return go

            def project_v(xT_s, sb):
                # V[s-slab, :] = x2_slab @ Wv into the vpp head blocks
                for q in range(4):
                    vproj_piece(sb, q)()

            # -- x2 -> K^T, V''; x1 transposes stream behind on the DMA.
            #    qproj0 runs before the last K slab so attention can start
            #    immediately after; vproj slab3 is deferred into the fill
            #    queue (its vpp rows are first read several units in) --
            for sb in range(n_slabs - 1):
                with nc.named_scope("kproj"):
                    project_jmajor(x2Ts[sb], wk_sb, sb, kT, bk_sb)
                with nc.named_scope("vproj"):
                    project_v(x2Ts[sb], sb)
                # x1T slab sb+1 reuses x2T slab sb's pool slot; emit its
                # DMA only after that slab's readers (kproj/vproj above)
                xpose(x1Ts[sb + 1], x1r, sb + 1)
            with nc.named_scope("qproj"):
                project_jmajor(x1Ts[0], wq_sb, 0, qT, bq_sb, use_act=True)

            ybounce = dram.tile([seq, D], F32, tag="yin")

            cts = {}
            pus = {}

            yts = {}

            def oproj_piece(sc, cT, s8, nck):
                def go():
                  with nc.named_scope("oproj"):
                    if nck == 0:
                        yts[(sc, s8)] = ysb.tile([P, D], F32, tag="y",
                                                 name=f"yt_{sc}_{s8}")
                    yt = yts[(sc, s8)]
                    last = sc == n_slabs - 1
                    py = psum_mm.tile([P, 512], F32, tag="mm",
                                      name=f"py_{sc}_{s8}_{nck}")
                    for jc in range(j_chunks):
                        nc.tensor.matmul(
                            py[:],
                            mm(cT[:, jc, s8 * P:(s8 + 1) * P]),
                            mm(wo_sb[:, jc, nck * 512:(nck + 1) * 512]),
                            start=(jc == 0), stop=(jc == j_chunks - 1))
                    if last and nck == 1:
                        # the scalar engine is drained of exps at the very
                        # end; splitting the final evictions across ACT+DVE
                        # shortens the tail
                        nc.scalar.copy(
                            yt[:, nck * 512:(nck + 1) * 512], py[:])
                    else:
                        nc.vector.tensor_copy(
                            yt[:, nck * 512:(nck + 1) * 512], py[:])
                    if nck == 1:
                        del yts[(sc, s8)]
                        si = sc * 4 + s8
                        if with_collective or sc > 0:
                            nc.sync.dma_start(
                                ybounce[si * P:(si + 1) * P, :], yt[:])
                        else:
                            # timed (no-collective) build: the final
                            # DRAM->DRAM copy stands in for the untimed
                            # ReduceScatter, so write the covered rows
                            # straight to the output
                            nc.sync.dma_start(
                                y_out[si * P:(si + 1) * P, :], yt[:])
                return go

            def emit_pv(sc, h, kcp, et):
              with nc.named_scope("attn"):
                jc, po = h // 2, (h % 2) * DH
                if kcp == 0:
                    pus[(sc, h)] = psum_u.tile([P, 512], F32, tag="u",
                                               name=f"pu_{sc}_{h}")
                pu = pus[(sc, h)]
                for dk in range(2):
                    kc = kcp * 2 + dk
                    nc.tensor.matmul(
                        pu[:],
                        mm(vpp[:, kc, h * P:(h + 1) * P]),
                        mm(et[:, dk * 512:(dk + 1) * 512]),
                        start=(kcp == 0 and dk == 0),
                        stop=(kcp == n_kcp - 1 and dk == 1))
                if kcp == n_kcp - 1:
                    cT = cts[sc]
                    rt = small.tile([DH, 512], F32, tag="rt",
                                    name=f"rt_{sc}_{h}")
                    nc.vector.reciprocal(rt[:], pu[DH:P, :])
                    nc.vector.tensor_mul(
                        r(cT[po:po + DH, jc, :]), pu[0:DH, :], rt[:])
                    del pus[(sc, h)]
                    if h == HPC - 1:
                        cT_done = cts.pop(sc)
                        for s8 in range(4):
                            for nck in range(2):
                                fill.append(
                                    oproj_piece(sc, cT_done, s8, nck))

            pend = []
            import collections as _c
            fill = _c.deque()

            def emit_attn_unit(sc, h, kcp):
              with nc.named_scope("attn"):
                if (h, kcp) == (0, 0):
                    cts[sc] = ct_pool.tile([P, j_chunks, 512], F32,
                                           tag="cT", name=f"cT_{sc}")
                jc, po = h // 2, (h % 2) * DH
                ps = psum_s.tile([P, 1024], F32, tag="s",
                                 name=f"ps_{sc}_{h}_{kcp}")
                for dk in range(2):
                    kc = kcp * 2 + dk
                    nc.tensor.matmul(
                        ps[:, dk * 512:(dk + 1) * 512],
                        mm(kT[po:po + DH, jc, kc * P:(kc + 1) * P]),
                        mm(qT[po:po + DH, jc, sc * 512:(sc + 1) * 512]),
                        start=True, stop=True)
                et = epool.tile([P, 1024], F32, tag="e",
                                name=f"et_{sc}_{h}_{kcp}")
                nc.scalar.activation(r(et[:]), ps[:], EXP, scale=0.125)
                pend.append((sc, h, kcp, et))
                if len(pend) > lag:
                    emit_pv(*pend.pop(0))

            # -- attention: 4 chunks of 512 queries. The next chunk's
            #    Q-projection and the previous chunk's out-projection are
            #    drip-fed from the fill queue, one ~850ns piece per unit,
            #    so the PE stays busy while ACT works through the exps --
            # slab3's K and V projections are drip-fed at the start of
            # attention (kT slab3 is first read at unit 6, vpp rows 12-15
            # at unit 6+lag), so the attention stream starts ~5us earlier
            kp3 = jproj_pieces(wk_sb, x2Ts[3], 3, kT, bk_sb,
                               "kproj", step=4)
            vp3 = [vproj_piece(3, q, pool=psum_mm, tag="mm")
                   for q in range(4)]
            for a, b in zip(kp3, vp3):
                fill.append(a)
                fill.append(b)
            for sc in range(n_slabs):
                if sc + 1 < n_slabs:
                    fill.extend(qproj_pieces(sc + 1))
                for h in range(HPC):
                    for kcp in range(n_kcp):
                        emit_attn_unit(sc, h, kcp)
                        u = h * n_kcp + kcp
                        if sc == 0 and u < 8 and u % 2 == 0:
                            # double-pop: slab3's deferred K/V projections
                            # must land before units 6..10 consume them
                            for _ in range(min(2, len(fill))):
                                fill.popleft()()
                        elif fill and (
                                u % 2 == 0 if sc < n_slabs - 1
                                else (u % 4 == 0 or h == HPC - 1)):
                            fill.popleft()()
            with nc.named_scope("attn"):
                for args in pend:
                    emit_pv(*args)
                    if fill:
                        fill.popleft()()
                while fill:
                    fill.popleft()()

            # -- sum partials across the 4 cores of this batch --
            # Two half-sized ReduceScatters: the first depends only on the
            # first 1024 rows, so it overlaps the second half's attention.
            if with_collective:
                half = seq // 2                 # 1024 rows per collective
                qr = seq // GROUPS // 2         # 256 rows per rank per half
                for ci in range(2):
                    ysc = dram.tile([qr, D], F32, tag="yout",
                                    name=f"ysc_{ci}")
                    nc.gpsimd.collective_compute(
                        "ReduceScatter",
                        mybir.AluOpType.add,
                        replica_groups=[[0, 1, 2, 3], [4, 5, 6, 7]],
                        ins=[ybounce[ci * half:(ci + 1) * half, :].opt()],
                        outs=[ysc[:].opt()],
                    )
                    nc.sync.dma_start(y_out[ci * qr:(ci + 1) * qr, :], ysc[:])
            # (no-collective build: y_out rows were written directly by
            # emit_oproj's sc==0 DMAs)

    nc.compile()
    return nc


def _get_program(seq=SEQ):
    if seq not in _cached:
        _cached[seq] = _build_program(seq)
    return _cached[seq]


def make_in_maps(x1, x2, Wq, bq, Wk, bk, Wv, bv, Wo, bo):
    """Per-core input dicts for the SPMD program (x and Wqkv host-cast to
    bf16; attention itself stays f32r on-chip)."""
    import ml_dtypes
    bf16 = ml_dtypes.bfloat16
    x1 = np.asarray(x1, np.float32).astype(bf16)
    x2 = np.asarray(x2, np.float32).astype(bf16)
    Wqh = np.asarray(Wq, np.float32).astype(bf16)
    Wkh = np.asarray(Wk, np.float32).astype(bf16)
    Wvh = np.asarray(Wv, np.float32).astype(bf16)
    Wo = np.asarray(Wo, np.float32)
    bq = np.asarray(bq, np.float32)
    bk = np.asarray(bk, np.float32)
    in_maps = []
    for c in range(N_CORES):
        b, g = c // GROUPS, c % GROUPS
        js = slice(g * JG, (g + 1) * JG)
        in_maps.append({
            "x1r": np.ascontiguousarray(x1[b]),
            "x2r": np.ascontiguousarray(x2[b]),
            "wq": np.ascontiguousarray(Wqh[:, js]),
            "wk": np.ascontiguousarray(Wkh[:, js]),
            "wv": np.ascontiguousarray(Wvh[:, js]),
            "wo": np.ascontiguousarray(Wo[js, :]),
            "bqr": np.ascontiguousarray(bq[js].reshape(2, P).T),
            "bkr": np.ascontiguousarray(bk[js].reshape(2, P).T),
        })
    return in_maps


def assemble(results, Wv_bias_fix):
    """results: list of per-core {'y_out': [seq//GROUPS, D]}.

    y_out rows [0:q) = rank's quarter of input rows [0:seq/2);
    rows [q:2q) = rank's quarter of input rows [seq/2:seq)."""
    seq = results[0]["y_out"].shape[0] * GROUPS
    q = seq // GROUPS // 2
    Y = np.empty((B, seq, D), np.float32)
    for c in range(N_CORES):
        b, rr = c // GROUPS, c % GROUPS
        yo = results[c]["y_out"]
        Y[b, rr * q:(rr + 1) * q, :] = yo[:q]
        Y[b, seq // 2 + rr * q:seq // 2 + (rr + 1) * q, :] = yo[q:]
    Y += Wv_bias_fix
    return Y


def kernel(x1, x2, Wq, bq, Wk, bk, Wv, bv, Wo, bo):
    from concourse.bass_utils import run_bass_kernel_spmd

    Wo = np.asarray(Wo, np.float32)
    bv = np.asarray(bv, np.float32)
    bo = np.asarray(bo, np.float32)

    nc = _get_program(SEQ)
    in_maps = make_in_maps(x1, x2, Wq, bq, Wk, bk, Wv, bv, Wo, bo)
    res = run_bass_kernel_spmd(nc, in_maps, core_ids=list(range(N_CORES)))
    fix = (bv @ Wo + bo).astype(np.float32)
    return assemble(res.results, fix)


# revision 27
# speedup vs baseline: 1.3005x; 1.0089x over previous
"""Multi-head cross-attention on 8 Trainium2 NeuronCores.

Sharding: data-parallel over batch (2) x tensor-parallel over heads (4 groups
of 4 heads). Core c handles batch c//4, head-group c%4 (a 256-wide slice of
the QKV projection space). Each core computes a partial output-projection
Y_partial = ctx_c @ Wo_c; a ReduceScatter(add) over each batch's 4 cores
leaves each core with a 512-row shard of the summed output, which the host
concatenates.

On-core dataflow:
  - x1/x2 arrive as bf16 (host-cast); x^T is produced by the DMA xbar
    (dma_start_transpose, 16x128 tiles) straight from DRAM -- the PE does no
    transposes at all. QKV projections run bf16 x bf16 into f32 PSUM.
  - Q^T/K^T = W.T @ x^T come out j-major, V = x @ Wv comes out s-major --
    exactly the operand layouts the attention matmuls need.
  - attention runs in f32r at full PE rate, tiled as (512-query chunk sc,
    head h, key-chunk pair): scores for two 128-key chunks land in one
    [128,1024] PSUM tile and are exp'd in a single scalar-engine op (no max
    subtraction: logits ~ N(0,1)). V carries 64 ones-columns so the softmax
    denominator accumulates in PSUM partitions 64..127 of the same PV
    matmul chain; one reciprocal+multiply normalizes into cT.
  - the PV stream lags the exp stream by a few units, and the next chunk's
    Q-projection plus the previous chunk's out-projection are emitted inside
    the attention stream so the PE never starves while the scalar engine
    works through the exps.
  - bq/bk are applied on-device (per-partition bias in j-major layout).
    bv/bo commute through softmax/out-projection exactly (softmax rows sum
    to 1), so the host adds bv @ Wo + bo to the final output.
  - a zero-matmul warms the PE p-state ramp during the initial DMA fill.
"""

import numpy as np

B, SEQ, D, H, DH = 2, 2048, 1024, 16, 64
N_CORES = 8
GROUPS = 4            # head-groups per batch (cores per batch)
JG = D // GROUPS      # 256 projection dims per core
HPC = H // GROUPS     # 4 heads per core
P = 128

_cached = {}


def _build_program(seq=SEQ, with_collective=True, lag=3):
    import concourse.tile as tile
    from concourse import bacc, mybir

    F32 = mybir.dt.float32
    BF16 = mybir.dt.bfloat16
    F32R = mybir.dt.float32r

    def r(x):
        return x.bitcast(F32R)

    mm = r  # matmul operands are f32r views of f32 tiles

    d_chunks = D // P            # 8
    j_chunks = JG // P           # 2
    n_slabs = seq // 512         # 4 (512-row x blocks and 512-query chunks)
    s_chunks = seq // P          # 16 (128-key chunks)
    n_kcp = s_chunks // 2        # 8 key-chunk pairs per (sc, h)

    nc = bacc.Bacc("TRN2", target_bir_lowering=False, debug=False,
                   num_devices=N_CORES)

    x1r = nc.dram_tensor("x1r", [seq, D], BF16, kind="ExternalInput")
    x2r = nc.dram_tensor("x2r", [seq, D], BF16, kind="ExternalInput")
    wq = nc.dram_tensor("wq", [D, JG], BF16, kind="ExternalInput")
    wk = nc.dram_tensor("wk", [D, JG], BF16, kind="ExternalInput")
    wv = nc.dram_tensor("wv", [D, JG], BF16, kind="ExternalInput")
    wo = nc.dram_tensor("wo", [JG, D], F32, kind="ExternalInput")
    bqr = nc.dram_tensor("bqr", [P, j_chunks], F32, kind="ExternalInput")
    bkr = nc.dram_tensor("bkr", [P, j_chunks], F32, kind="ExternalInput")
    y_out = nc.dram_tensor("y_out", [seq // GROUPS, D], F32,
                           kind="ExternalOutput")

    EXP = mybir.ActivationFunctionType.Exp

    with tile.TileContext(nc) as tc:
        with (
            tc.tile_pool(name="consts", bufs=1) as consts,
            tc.tile_pool(name="wqkv", bufs=3) as wqkv_pool,
            tc.tile_pool(name="wop", bufs=1) as wo_pool,
            tc.tile_pool(name="xt", bufs=5) as xt_pool,
            tc.tile_pool(name="acts", bufs=1) as acts,
            tc.tile_pool(name="ctp", bufs=2) as ct_pool,
            tc.tile_pool(name="epool", bufs=4) as epool,
            tc.tile_pool(name="small", bufs=2) as small,
            tc.tile_pool(name="ysb", bufs=4) as ysb,
            tc.tile_pool(name="psum_mm", bufs=1, space="PSUM") as psum_mm,
            tc.tile_pool(name="psum_q", bufs=1, space="PSUM") as psum_q,
            tc.tile_pool(name="psum_s", bufs=2, space="PSUM") as psum_s,
            tc.tile_pool(name="psum_u", bufs=2, space="PSUM") as psum_u,
            tc.tile_pool(name="dram", bufs=1, space="DRAM") as dram,
        ):
            # PE p-state warmup: dummy matmuls spread out by ping-ponging
            # through a DVE copy (two semaphore hops each, ~400ns apart) so
            # the tensor engine never idles long enough to reset its clock
            # ramp while the initial DMAs fill SBUF.
            zt = consts.tile([P, P], BF16, tag="warm")
            nc.gpsimd.memset(zt[:], 0.0)
            wsb = consts.tile([P, 16], F32, tag="warm2")
            pwarm = psum_mm.tile([P, 512], F32, tag="mm", name="pwarm")
            for _ in range(11):
                nc.tensor.matmul(pwarm[:, 0:16], zt[:], zt[:, 0:16],
                                 start=True, stop=True)
                nc.vector.tensor_copy(wsb[:], pwarm[:, 0:16])

            def xpose2(dst, x_dram, sb):
                # finer (2-block) pieces: lower first-chunk latency
                for g in range(d_chunks // 2):
                    nc.sync.dma_start_transpose(
                        dst[:, 2 * g:2 * (g + 1), :],
                        x_dram[sb * 512:(sb + 1) * 512,
                               g * 256:(g + 1) * 256])

            def xpose(dst, x_dram, sb):
                # x rows [sb*512,(sb+1)*512) -> dst[:, dc, :] = slab^T (bf16).
                # One xbar instruction transposes four 128-col blocks into the
                # 3D [128, 4, 512] layout directly (in [512, 4*128] reshaped
                # (512,4,128) then reversed-transposed is exactly d-major).
                for g in range(d_chunks // 4):
                    nc.sync.dma_start_transpose(
                        dst[:, 4 * g:4 * (g + 1), :],
                        x_dram[sb * 512:(sb + 1) * 512,
                               g * 512:(g + 1) * 512])

            # -- DMA order: wk first (first kproj needs it), then x2 slab0
            #    transposes so kproj starts ASAP --
            x2Ts = [xt_pool.tile([P, d_chunks, 512], BF16, tag="xT",
                                 name=f"x2T_{sb}") for sb in range(n_slabs)]
            wk_sb = wqkv_pool.tile([P, d_chunks, JG], BF16, tag="wqkv")
            wv_sb = wqkv_pool.tile([P, d_chunks, JG], BF16, tag="wqkv")
            wq_sb = wqkv_pool.tile([P, d_chunks, JG], BF16, tag="wqkv")
            nc.sync.dma_start(wk_sb[:],
                              wk.rearrange("(o p) j -> p o j", p=P))
            nc.sync.dma_start(wv_sb[:],
                              wv.rearrange("(o p) j -> p o j", p=P))
            bq_sb = consts.tile([P, j_chunks], F32, tag="bq")
            bk_sb = consts.tile([P, j_chunks], F32, tag="bk")
            nc.sync.dma_start(bq_sb[:], bqr[:])
            nc.sync.dma_start(bk_sb[:], bkr[:])
            x1Ts = [xt_pool.tile([P, d_chunks, 512], BF16, tag="xT",
                                 name=f"x1T_{sb}") for sb in range(n_slabs)]
            xpose2(x2Ts[0], x2r, 0)
            for sb in range(1, n_slabs):
                xpose(x2Ts[sb], x2r, sb)
            xpose(x1Ts[0], x1r, 0)
            nc.sync.dma_start(wq_sb[:],
                              wq.rearrange("(o p) j -> p o j", p=P))
            wo_sb = wo_pool.tile([P, j_chunks, D], F32, tag="wo")
            for o in range(j_chunks):
                st = ysb.tile([P, D], F32, tag="y", name=f"wst_{o}")
                nc.sync.dma_start(
                    st[:], wo.rearrange("(o p) n -> p o n", p=P)[:, o, :])
                nc.vector.tensor_copy(r(wo_sb[:, o, :]), st[:])

            # -- persistent activations --
            kT = acts.tile([P, j_chunks, seq], F32, tag="kT")
            qT = acts.tile([P, j_chunks, seq], F32, tag="qT")
            # V'' per head-column-block: cols 0..63 V_h, 64..127 ones
            vpp = acts.tile([P, s_chunks, HPC * P], F32, tag="vpp")

            ones_f32 = consts.tile([P, DH], F32, tag="ones")
            nc.vector.memset(ones_f32[:], 1.0)
            for si in range(s_chunks):
                ones_view = vpp[:, si].rearrange(
                    "p (h q) -> p h q", q=P)[:, :, DH:P]
                # scalar engine is idle before attention; it also rounds f32r
                nc.scalar.copy(
                    r(ones_view),
                    ones_f32[:, None, :].to_broadcast([P, HPC, DH]))

            def project_jmajor(xT_s, w_sb, sb, out, bias, use_act=False):
                # out[:, jc, sb-slab] = w.T @ x^T + bias (j-major); the two
                # jc chains use separate single-buffer pools so they overlap
                for jc in range(j_chunks):
                    pool = psum_q if jc == 0 else psum_mm
                    pk = pool.tile([P, 512], F32,
                                   tag=("q" if jc == 0 else "mm"),
                                   name=f"pk_{w_sb.name}_{sb}_{jc}")
                    for dc in range(d_chunks):
                        nc.tensor.matmul(
                            pk[:],
                            w_sb[:, dc, jc * P:(jc + 1) * P],
                            xT_s[:, dc, :],
                            start=(dc == 0), stop=(dc == d_chunks - 1))
                    if use_act:
                        nc.scalar.add(
                            r(out[:, jc, sb * 512:(sb + 1) * 512]),
                            pk[:], bias[:, jc:jc + 1])
                    else:
                        nc.vector.tensor_scalar_add(
                            r(out[:, jc, sb * 512:(sb + 1) * 512]),
                            pk[:], bias[:, jc:jc + 1])

            def jproj_pieces(w_sb, xT, sb, out, bias, scope, step=2):
                # j-major projection split into ~425ns closures drip-fed
                # between attention units; the dedicated single-buffer
                # psum_q pool holds the open accumulation chain (the two jc
                # chains run back to back, never concurrently)
                state = {}

                def piece(jc, lo):
                    def go():
                      with nc.named_scope(scope):
                        if lo == 0:
                            state[jc] = psum_q.tile(
                                [P, 512], F32, tag="q",
                                name=f"pj_{scope}_{sb}_{jc}")
                        pk = state[jc]
                        for dc in range(lo, lo + step):
                            nc.tensor.matmul(
                                pk[:],
                                w_sb[:, dc, jc * P:(jc + 1) * P],
                                xT[:, dc, :],
                                start=(dc == 0), stop=(dc == d_chunks - 1))
                        if lo + step == d_chunks:
                            nc.vector.tensor_scalar_add(
                                r(out[:, jc, sb * 512:(sb + 1) * 512]),
                                pk[:], bias[:, jc:jc + 1])
                    return go

                return [piece(jc, lo) for jc in range(j_chunks)
                        for lo in range(0, d_chunks, step)]

            def qproj_pieces(sb):
                return jproj_pieces(wq_sb, x1Ts[sb], sb, qT, bq_sb,
                                    "qproj", step=2)

            def vproj_piece(sb, q, pool=None, tag="u"):
                # fill-time pieces must NOT use psum_u: its round-robin slot
                # may hold a live PV accumulator mid-attention
                def go():
                  with nc.named_scope("vproj"):
                    si = sb * 4 + q
                    pv = (pool or psum_u).tile([P, 512], F32, tag=tag,
                                               name=f"pv_{si}")
                    for dc in range(d_chunks):
                        nc.tensor.matmul(
                            pv[:, 0:JG],
                            x2Ts[sb][:, dc, q * P:(q + 1) * P],
                            wv_sb[:, dc, :],
                            start=(dc == 0), stop=(dc == d_chunks - 1))
                    vv = vpp[:, si].rearrange(
                        "p (h q) -> p h q", q=P)[:, :, 0:DH]
                    nc.vector.tensor_copy(
                        r(vv),
                        pv[:, 0:JG].rearrange("p (h q) -> p h q", q=DH))
                return go

            def project_v(xT_s, sb):
                # V[s-slab, :] = x2_slab @ Wv into the vpp head blocks
                for q in range(4):
                    vproj_piece(sb, q)()

            # -- x2 -> K^T, V''; x1 transposes stream behind on the DMA.
            #    qproj0 runs before the last K slab so attention can start
            #    immediately after; vproj slab3 is deferred into the fill
            #    queue (its vpp rows are first read several units in) --
            for sb in range(n_slabs - 1):
                with nc.named_scope("kproj"):
                    project_jmajor(x2Ts[sb], wk_sb, sb, kT, bk_sb)
                with nc.named_scope("vproj"):
                    project_v(x2Ts[sb], sb)
                # x1T slab sb+1 reuses x2T slab sb's pool slot; emit its
                # DMA only after that slab's readers (kproj/vproj above)
                xpose(x1Ts[sb + 1], x1r, sb + 1)
            with nc.named_scope("qproj"):
                project_jmajor(x1Ts[0], wq_sb, 0, qT, bq_sb, use_act=True)

            ybounce = dram.tile([seq, D], F32, tag="yin")

            cts = {}
            pus = {}

            yts = {}

            def oproj_piece(sc, cT, s8, nck):
                def go():
                  with nc.named_scope("oproj"):
                    if nck == 0:
                        yts[(sc, s8)] = ysb.tile([P, D], F32, tag="y",
                                                 name=f"yt_{sc}_{s8}")
                    yt = yts[(sc, s8)]
                    last = sc == n_slabs - 1
                    py = psum_mm.tile([P, 512], F32, tag="mm",
                                      name=f"py_{sc}_{s8}_{nck}")
                    for jc in range(j_chunks):
                        nc.tensor.matmul(
                            py[:],
                            mm(cT[:, jc, s8 * P:(s8 + 1) * P]),
                            mm(wo_sb[:, jc, nck * 512:(nck + 1) * 512]),
                            start=(jc == 0), stop=(jc == j_chunks - 1))
                    csl = slice(nck * 512, (nck + 1) * 512)
                    if last and nck == 1:
                        # the scalar engine is drained of exps at the very
                        # end; splitting the final evictions across ACT+DVE
                        # shortens the tail
                        nc.scalar.copy(yt[:, csl], py[:])
                    else:
                        nc.vector.tensor_copy(yt[:, csl], py[:])
                    si = sc * 4 + s8
                    # half-row DMAs pipeline with the eviction stream
                    if with_collective or sc > 0:
                        nc.sync.dma_start(
                            ybounce[si * P:(si + 1) * P, csl], yt[:, csl])
                    else:
                        # timed (no-collective) build: the final DRAM->DRAM
                        # copy stands in for the untimed ReduceScatter, so
                        # write the covered rows straight to the output
                        nc.sync.dma_start(
                            y_out[si * P:(si + 1) * P, csl], yt[:, csl])
                    if nck == 1:
                        del yts[(sc, s8)]
                return go

            def emit_pv(sc, h, kcp, et):
              with nc.named_scope("attn"):
                jc, po = h // 2, (h % 2) * DH
                if kcp == 0:
                    pus[(sc, h)] = psum_u.tile([P, 512], F32, tag="u",
                                               name=f"pu_{sc}_{h}")
                pu = pus[(sc, h)]
                for dk in range(2):
                    kc = kcp * 2 + dk
                    nc.tensor.matmul(
                        pu[:],
                        mm(vpp[:, kc, h * P:(h + 1) * P]),
                        mm(et[:, dk * 512:(dk + 1) * 512]),
                        start=(kcp == 0 and dk == 0),
                        stop=(kcp == n_kcp - 1 and dk == 1))
                if kcp == n_kcp - 1:
                    cT = cts[sc]
                    rt = small.tile([DH, 512], F32, tag="rt",
                                    name=f"rt_{sc}_{h}")
                    nc.vector.reciprocal(rt[:], pu[DH:P, :])
                    nc.vector.tensor_mul(
                        r(cT[po:po + DH, jc, :]), pu[0:DH, :], rt[:])
                    del pus[(sc, h)]
                    if h == HPC - 1:
                        cT_done = cts.pop(sc)
                        for s8 in range(4):
                            for nck in range(2):
                                fill.append(
                                    oproj_piece(sc, cT_done, s8, nck))

            pend = []
            import collections as _c
            fill = _c.deque()

            def emit_attn_unit(sc, h, kcp):
              with nc.named_scope("attn"):
                if (h, kcp) == (0, 0):
                    cts[sc] = ct_pool.tile([P, j_chunks, 512], F32,
                                           tag="cT", name=f"cT_{sc}")
                jc, po = h // 2, (h % 2) * DH
                ps = psum_s.tile([P, 1024], F32, tag="s",
                                 name=f"ps_{sc}_{h}_{kcp}")
                for dk in range(2):
                    kc = kcp * 2 + dk
                    nc.tensor.matmul(
                        ps[:, dk * 512:(dk + 1) * 512],
                        mm(kT[po:po + DH, jc, kc * P:(kc + 1) * P]),
                        mm(qT[po:po + DH, jc, sc * 512:(sc + 1) * 512]),
                        start=True, stop=True)
                et = epool.tile([P, 1024], F32, tag="e",
                                name=f"et_{sc}_{h}_{kcp}")
                nc.scalar.activation(r(et[:]), ps[:], EXP, scale=0.125)
                pend.append((sc, h, kcp, et))
                if len(pend) > lag:
                    emit_pv(*pend.pop(0))

            # -- attention: 4 chunks of 512 queries. The next chunk's
            #    Q-projection and the previous chunk's out-projection are
            #    drip-fed from the fill queue, one ~850ns piece per unit,
            #    so the PE stays busy while ACT works through the exps --
            # slab3's K and V projections are drip-fed at the start of
            # attention (kT slab3 is first read at unit 6, vpp rows 12-15
            # at unit 6+lag), so the attention stream starts ~5us earlier
            kp3 = jproj_pieces(wk_sb, x2Ts[3], 3, kT, bk_sb,
                               "kproj", step=4)
            vp3 = [vproj_piece(3, q, pool=psum_mm, tag="mm")
                   for q in range(4)]
            for a, b in zip(kp3, vp3):
                fill.append(a)
                fill.append(b)
            for sc in range(n_slabs):
                if sc + 1 < n_slabs:
                    fill.extend(qproj_pieces(sc + 1))
                for h in range(HPC):
                    for kcp in range(n_kcp):
                        emit_attn_unit(sc, h, kcp)
                        u = h * n_kcp + kcp
                        if sc == 0 and u < 8 and u % 2 == 0:
                            # double-pop: slab3's deferred K/V projections
                            # must land before units 6..10 consume them
                            for _ in range(min(2, len(fill))):
                                fill.popleft()()
                        elif fill and (
                                u % 2 == 0 if sc < n_slabs - 1
                                else h == HPC - 1):
                            fill.popleft()()
            with nc.named_scope("attn"):
                for args in pend:
                    emit_pv(*args)
                    if fill:
                        fill.popleft()()
                while fill:
                    fill.popleft()()

            # -- sum partials across the 4 cores of this batch --
            # Two half-sized ReduceScatters: the first depends only on the
            # first 1024 rows, so it overlaps the second half's attention.
            if with_collective:
                half = seq // 2                 # 1024 rows per collective
                qr = seq // GROUPS // 2         # 256 rows per rank per half
                for ci in range(2):
                    ysc = dram.tile([qr, D], F32, tag="yout",
                                    name=f"ysc_{ci}")
                    nc.gpsimd.collective_compute(
                        "ReduceScatter",
                        mybir.AluOpType.add,
                        replica_groups=[[0, 1, 2, 3], [4, 5, 6, 7]],
                        ins=[ybounce[ci * half:(ci + 1) * half, :].opt()],
                        outs=[ysc[:].opt()],
                    )
                    nc.sync.dma_start(y_out[ci * qr:(ci + 1) * qr, :], ysc[:])
            # (no-collective build: y_out rows were written directly by
            # emit_oproj's sc==0 DMAs)

    nc.compile()
    return nc


def _get_program(seq=SEQ):
    if seq not in _cached:
        _cached[seq] = _build_program(seq)
    return _cached[seq]


def make_in_maps(x1, x2, Wq, bq, Wk, bk, Wv, bv, Wo, bo):
    """Per-core input dicts for the SPMD program (x and Wqkv host-cast to
    bf16; attention itself stays f32r on-chip)."""
    import ml_dtypes
    bf16 = ml_dtypes.bfloat16
    x1 = np.asarray(x1, np.float32).astype(bf16)
    x2 = np.asarray(x2, np.float32).astype(bf16)
    Wqh = np.asarray(Wq, np.float32).astype(bf16)
    Wkh = np.asarray(Wk, np.float32).astype(bf16)
    Wvh = np.asarray(Wv, np.float32).astype(bf16)
    Wo = np.asarray(Wo, np.float32)
    bq = np.asarray(bq, np.float32)
    bk = np.asarray(bk, np.float32)
    in_maps = []
    for c in range(N_CORES):
        b, g = c // GROUPS, c % GROUPS
        js = slice(g * JG, (g + 1) * JG)
        in_maps.append({
            "x1r": np.ascontiguousarray(x1[b]),
            "x2r": np.ascontiguousarray(x2[b]),
            "wq": np.ascontiguousarray(Wqh[:, js]),
            "wk": np.ascontiguousarray(Wkh[:, js]),
            "wv": np.ascontiguousarray(Wvh[:, js]),
            "wo": np.ascontiguousarray(Wo[js, :]),
            "bqr": np.ascontiguousarray(bq[js].reshape(2, P).T),
            "bkr": np.ascontiguousarray(bk[js].reshape(2, P).T),
        })
    return in_maps


def assemble(results, Wv_bias_fix):
    """results: list of per-core {'y_out': [seq//GROUPS, D]}.

    y_out rows [0:q) = rank's quarter of input rows [0:seq/2);
    rows [q:2q) = rank's quarter of input rows [seq/2:seq)."""
    seq = results[0]["y_out"].shape[0] * GROUPS
    q = seq // GROUPS // 2
    Y = np.empty((B, seq, D), np.float32)
    for c in range(N_CORES):
        b, rr = c // GROUPS, c % GROUPS
        yo = results[c]["y_out"]
        Y[b, rr * q:(rr + 1) * q, :] = yo[:q]
        Y[b, seq // 2 + rr * q:seq // 2 + (rr + 1) * q, :] = yo[q:]
    Y += Wv_bias_fix
    return Y


def kernel(x1, x2, Wq, bq, Wk, bk, Wv, bv, Wo, bo):
    from concourse.bass_utils import run_bass_kernel_spmd

    Wo = np.asarray(Wo, np.float32)
    bv = np.asarray(bv, np.float32)
    bo = np.asarray(bo, np.float32)

    nc = _get_program(SEQ)
    in_maps = make_in_maps(x1, x2, Wq, bq, Wk, bk, Wv, bv, Wo, bo)
    res = run_bass_kernel_spmd(nc, in_maps, core_ids=list(range(N_CORES)))
    fix = (bv @ Wo + bo).astype(np.float32)
    return assemble(res.results, fix)


# revision 30
# speedup vs baseline: 1.3295x; 1.0223x over previous
"""Multi-head cross-attention on 8 Trainium2 NeuronCores.

Sharding: data-parallel over batch (2) x tensor-parallel over heads (4 groups
of 4 heads). Core c handles batch c//4, head-group c%4 (a 256-wide slice of
the QKV projection space). Each core computes a partial output-projection
Y_partial = ctx_c @ Wo_c; a ReduceScatter(add) over each batch's 4 cores
leaves each core with a 512-row shard of the summed output, which the host
concatenates.

On-core dataflow:
  - x1/x2 arrive as bf16 (host-cast); x^T is produced by the DMA xbar
    (dma_start_transpose, 16x128 tiles) straight from DRAM -- the PE does no
    transposes at all. QKV projections run bf16 x bf16 into f32 PSUM.
  - Q^T/K^T = W.T @ x^T come out j-major, V = x @ Wv comes out s-major --
    exactly the operand layouts the attention matmuls need.
  - attention runs in f32r at full PE rate, tiled as (512-query chunk sc,
    head h, key-chunk pair): scores for two 128-key chunks land in one
    [128,1024] PSUM tile and are exp'd in a single scalar-engine op (no max
    subtraction: logits ~ N(0,1)). V carries 64 ones-columns so the softmax
    denominator accumulates in PSUM partitions 64..127 of the same PV
    matmul chain; one reciprocal+multiply normalizes into cT.
  - the PV stream lags the exp stream by a few units, and the next chunk's
    Q-projection plus the previous chunk's out-projection are emitted inside
    the attention stream so the PE never starves while the scalar engine
    works through the exps.
  - bq/bk are applied on-device (per-partition bias in j-major layout).
    bv/bo commute through softmax/out-projection exactly (softmax rows sum
    to 1), so the host adds bv @ Wo + bo to the final output.
  - a zero-matmul warms the PE p-state ramp during the initial DMA fill.
"""

import numpy as np

B, SEQ, D, H, DH = 2, 2048, 1024, 16, 64
N_CORES = 8
GROUPS = 4            # head-groups per batch (cores per batch)
JG = D // GROUPS      # 256 projection dims per core
HPC = H // GROUPS     # 4 heads per core
P = 128

_cached = {}


def _build_program(seq=SEQ, with_collective=True, lag=3):
    import concourse.tile as tile
    from concourse import bacc, mybir

    F32 = mybir.dt.float32
    BF16 = mybir.dt.bfloat16
    F32R = mybir.dt.float32r

    def r(x):
        return x.bitcast(F32R)

    mm = r  # matmul operands are f32r views of f32 tiles

    d_chunks = D // P            # 8
    j_chunks = JG // P           # 2
    n_slabs = seq // 512         # 4 (512-row x blocks and 512-query chunks)
    s_chunks = seq // P          # 16 (128-key chunks)
    n_kcp = s_chunks // 2        # 8 key-chunk pairs per (sc, h)

    nc = bacc.Bacc("TRN2", target_bir_lowering=False, debug=False,
                   num_devices=N_CORES)

    x1r = nc.dram_tensor("x1r", [seq, D], BF16, kind="ExternalInput")
    x2r = nc.dram_tensor("x2r", [seq, D], BF16, kind="ExternalInput")
    wq = nc.dram_tensor("wq", [D, JG], BF16, kind="ExternalInput")
    wk = nc.dram_tensor("wk", [D, JG], BF16, kind="ExternalInput")
    wv = nc.dram_tensor("wv", [D, JG], BF16, kind="ExternalInput")
    wo = nc.dram_tensor("wo", [JG, D], F32, kind="ExternalInput")
    bqr = nc.dram_tensor("bqr", [P, j_chunks], F32, kind="ExternalInput")
    bkr = nc.dram_tensor("bkr", [P, j_chunks], F32, kind="ExternalInput")
    y_out = nc.dram_tensor("y_out", [seq // GROUPS, D], F32,
                           kind="ExternalOutput")

    EXP = mybir.ActivationFunctionType.Exp

    with tile.TileContext(nc) as tc:
        with (
            tc.tile_pool(name="consts", bufs=1) as consts,
            tc.tile_pool(name="wqkv", bufs=3) as wqkv_pool,
            tc.tile_pool(name="wop", bufs=1) as wo_pool,
            tc.tile_pool(name="xt", bufs=5) as xt_pool,
            tc.tile_pool(name="acts", bufs=1) as acts,
            tc.tile_pool(name="ctp", bufs=2) as ct_pool,
            tc.tile_pool(name="epool", bufs=4) as epool,
            tc.tile_pool(name="small", bufs=2) as small,
            tc.tile_pool(name="ysb", bufs=4) as ysb,
            tc.tile_pool(name="psum_mm", bufs=1, space="PSUM") as psum_mm,
            tc.tile_pool(name="psum_q", bufs=1, space="PSUM") as psum_q,
            tc.tile_pool(name="psum_s", bufs=2, space="PSUM") as psum_s,
            tc.tile_pool(name="psum_u", bufs=2, space="PSUM") as psum_u,
            tc.tile_pool(name="dram", bufs=1, space="DRAM") as dram,
        ):
            # PE p-state warmup: dummy matmuls spread out by ping-ponging
            # through a DVE copy (two semaphore hops each, ~400ns apart) so
            # the tensor engine never idles long enough to reset its clock
            # ramp while the initial DMAs fill SBUF.
            zt = consts.tile([P, P], BF16, tag="warm")
            nc.gpsimd.memset(zt[:], 0.0)
            wsb = consts.tile([P, 16], F32, tag="warm2")
            pwarm = psum_mm.tile([P, 512], F32, tag="mm", name="pwarm")
            for _ in range(11):
                nc.tensor.matmul(pwarm[:, 0:16], zt[:], zt[:, 0:16],
                                 start=True, stop=True)
                nc.vector.tensor_copy(wsb[:], pwarm[:, 0:16])

            def xpose2(dst, x_dram, sb):
                # finer (2-block) pieces: lower first-chunk latency
                for g in range(d_chunks // 2):
                    nc.sync.dma_start_transpose(
                        dst[:, 2 * g:2 * (g + 1), :],
                        x_dram[sb * 512:(sb + 1) * 512,
                               g * 256:(g + 1) * 256])

            def xpose(dst, x_dram, sb):
                # x rows [sb*512,(sb+1)*512) -> dst[:, dc, :] = slab^T (bf16).
                # One xbar instruction transposes four 128-col blocks into the
                # 3D [128, 4, 512] layout directly (in [512, 4*128] reshaped
                # (512,4,128) then reversed-transposed is exactly d-major).
                for g in range(d_chunks // 4):
                    nc.sync.dma_start_transpose(
                        dst[:, 4 * g:4 * (g + 1), :],
                        x_dram[sb * 512:(sb + 1) * 512,
                               g * 512:(g + 1) * 512])

            # -- DMA order: wk first (first kproj needs it), then x2 slab0
            #    transposes so kproj starts ASAP --
            x2Ts = [xt_pool.tile([P, d_chunks, 512], BF16, tag="xT",
                                 name=f"x2T_{sb}") for sb in range(n_slabs)]
            wk_sb = wqkv_pool.tile([P, d_chunks, JG], BF16, tag="wqkv")
            wv_sb = wqkv_pool.tile([P, d_chunks, JG], BF16, tag="wqkv")
            wq_sb = wqkv_pool.tile([P, d_chunks, JG], BF16, tag="wqkv")
            nc.sync.dma_start(wk_sb[:],
                              wk.rearrange("(o p) j -> p o j", p=P))
            nc.sync.dma_start(wv_sb[:],
                              wv.rearrange("(o p) j -> p o j", p=P))
            bq_sb = consts.tile([P, j_chunks], F32, tag="bq")
            bk_sb = consts.tile([P, j_chunks], F32, tag="bk")
            nc.sync.dma_start(bq_sb[:], bqr[:])
            nc.sync.dma_start(bk_sb[:], bkr[:])
            x1Ts = [xt_pool.tile([P, d_chunks, 512], BF16, tag="xT",
                                 name=f"x1T_{sb}") for sb in range(n_slabs)]
            xpose2(x2Ts[0], x2r, 0)
            for sb in range(1, n_slabs):
                xpose(x2Ts[sb], x2r, sb)
            xpose(x1Ts[0], x1r, 0)
            nc.sync.dma_start(wq_sb[:],
                              wq.rearrange("(o p) j -> p o j", p=P))
            wo_sb = wo_pool.tile([P, j_chunks, D], F32, tag="wo")
            for o in range(j_chunks):
                st = ysb.tile([P, D], F32, tag="y", name=f"wst_{o}")
                nc.sync.dma_start(
                    st[:], wo.rearrange("(o p) n -> p o n", p=P)[:, o, :])
                nc.vector.tensor_copy(r(wo_sb[:, o, :]), st[:])

            # -- persistent activations --
            kT = acts.tile([P, j_chunks, seq], F32, tag="kT")
            qT = acts.tile([P, j_chunks, seq], F32, tag="qT")
            # V'' per head-column-block: cols 0..63 V_h, 64..127 ones
            vpp = acts.tile([P, s_chunks, HPC * P], F32, tag="vpp")

            ones_f32 = consts.tile([P, DH], F32, tag="ones")
            nc.vector.memset(ones_f32[:], 1.0)
            for si in range(s_chunks):
                ones_view = vpp[:, si].rearrange(
                    "p (h q) -> p h q", q=P)[:, :, DH:P]
                # scalar engine is idle before attention; it also rounds f32r
                nc.scalar.copy(
                    r(ones_view),
                    ones_f32[:, None, :].to_broadcast([P, HPC, DH]))

            def project_jmajor(xT_s, w_sb, sb, out, bias, use_act=False):
                # out[:, jc, sb-slab] = w.T @ x^T + bias (j-major); the two
                # jc chains use separate single-buffer pools so they overlap
                for jc in range(j_chunks):
                    pool = psum_q if jc == 0 else psum_mm
                    pk = pool.tile([P, 512], F32,
                                   tag=("q" if jc == 0 else "mm"),
                                   name=f"pk_{w_sb.name}_{sb}_{jc}")
                    for dc in range(d_chunks):
                        nc.tensor.matmul(
                            pk[:],
                            w_sb[:, dc, jc * P:(jc + 1) * P],
                            xT_s[:, dc, :],
                            start=(dc == 0), stop=(dc == d_chunks - 1))
                    if use_act:
                        nc.scalar.add(
                            r(out[:, jc, sb * 512:(sb + 1) * 512]),
                            pk[:], bias[:, jc:jc + 1])
                    else:
                        nc.vector.tensor_scalar_add(
                            r(out[:, jc, sb * 512:(sb + 1) * 512]),
                            pk[:], bias[:, jc:jc + 1])

            def jproj_pieces(w_sb, xT, sb, out, bias, scope, step=2):
                # j-major projection split into ~425ns closures drip-fed
                # between attention units; the dedicated single-buffer
                # psum_q pool holds the open accumulation chain (the two jc
                # chains run back to back, never concurrently)
                state = {}

                def piece(jc, lo):
                    def go():
                      with nc.named_scope(scope):
                        if lo == 0:
                            state[jc] = psum_q.tile(
                                [P, 512], F32, tag="q",
                                name=f"pj_{scope}_{sb}_{jc}")
                        pk = state[jc]
                        for dc in range(lo, lo + step):
                            nc.tensor.matmul(
                                pk[:],
                                w_sb[:, dc, jc * P:(jc + 1) * P],
                                xT[:, dc, :],
                                start=(dc == 0), stop=(dc == d_chunks - 1))
                        if lo + step == d_chunks:
                            nc.vector.tensor_scalar_add(
                                r(out[:, jc, sb * 512:(sb + 1) * 512]),
                                pk[:], bias[:, jc:jc + 1])
                    return go

                return [piece(jc, lo) for jc in range(j_chunks)
                        for lo in range(0, d_chunks, step)]

            def qproj_pieces(sb):
                return jproj_pieces(wq_sb, x1Ts[sb], sb, qT, bq_sb,
                                    "qproj", step=2)

            def vproj_piece(sb, q, pool=None, tag="u"):
                # fill-time pieces must NOT use psum_u: its round-robin slot
                # may hold a live PV accumulator mid-attention
                def go():
                  with nc.named_scope("vproj"):
                    si = sb * 4 + q
                    pv = (pool or psum_u).tile([P, 512], F32, tag=tag,
                                               name=f"pv_{si}")
                    for dc in range(d_chunks):
                        nc.tensor.matmul(
                            pv[:, 0:JG],
                            x2Ts[sb][:, dc, q * P:(q + 1) * P],
                            wv_sb[:, dc, :],
                            start=(dc == 0), stop=(dc == d_chunks - 1))
                    vv = vpp[:, si].rearrange(
                        "p (h q) -> p h q", q=P)[:, :, 0:DH]
                    nc.vector.tensor_copy(
                        r(vv),
                        pv[:, 0:JG].rearrange("p (h q) -> p h q", q=DH))
                return go

            def project_v(xT_s, sb):
                # V[s-slab, :] = x2_slab @ Wv into the vpp head blocks
                for q in range(4):
                    vproj_piece(sb, q)()

            # -- x2 -> K^T, V''; x1 transposes stream behind on the DMA.
            #    qproj0 runs before the last K slab so attention can start
            #    immediately after; vproj slab3 is deferred into the fill
            #    queue (its vpp rows are first read several units in) --
            for sb in range(n_slabs - 1):
                with nc.named_scope("kproj"):
                    project_jmajor(x2Ts[sb], wk_sb, sb, kT, bk_sb)
                with nc.named_scope("vproj"):
                    project_v(x2Ts[sb], sb)
                # x1T slab sb+1 reuses x2T slab sb's pool slot; emit its
                # DMA only after that slab's readers (kproj/vproj above)
                xpose(x1Ts[sb + 1], x1r, sb + 1)
            with nc.named_scope("qproj"):
                project_jmajor(x1Ts[0], wq_sb, 0, qT, bq_sb, use_act=True)

            ybounce = dram.tile([seq, D], F32, tag="yin")

            cts = {}
            pus = {}

            yts = {}

            def oproj_piece(sc, cT, s8, nck):
                def go():
                  with nc.named_scope("oproj"):
                    if nck == 0:
                        yts[(sc, s8)] = ysb.tile([P, D], F32, tag="y",
                                                 name=f"yt_{sc}_{s8}")
                    yt = yts[(sc, s8)]
                    last = sc == n_slabs - 1
                    # pieces for the last two chunks pop back-to-back in the
                    # final drain where psum_q is free; alternating pools
                    # breaks the matmul->evict->matmul serialization
                    if sc >= 2 and (s8 * 2 + nck) % 2:
                        py = psum_q.tile([P, 512], F32, tag="q",
                                         name=f"py_{sc}_{s8}_{nck}")
                    else:
                        py = psum_mm.tile([P, 512], F32, tag="mm",
                                          name=f"py_{sc}_{s8}_{nck}")
                    for jc in range(j_chunks):
                        nc.tensor.matmul(
                            py[:],
                            mm(cT[:, jc, s8 * P:(s8 + 1) * P]),
                            mm(wo_sb[:, jc, nck * 512:(nck + 1) * 512]),
                            start=(jc == 0), stop=(jc == j_chunks - 1))
                    csl = slice(nck * 512, (nck + 1) * 512)
                    if last and nck == 1:
                        # the scalar engine is drained of exps at the very
                        # end; splitting the final evictions across ACT+DVE
                        # shortens the tail
                        nc.scalar.copy(yt[:, csl], py[:])
                    else:
                        nc.vector.tensor_copy(yt[:, csl], py[:])
                    si = sc * 4 + s8
                    # half-row DMAs pipeline with the eviction stream
                    if with_collective or sc > 0:
                        nc.sync.dma_start(
                            ybounce[si * P:(si + 1) * P, csl], yt[:, csl])
                    else:
                        # timed (no-collective) build: the final DRAM->DRAM
                        # copy stands in for the untimed ReduceScatter, so
                        # write the covered rows straight to the output
                        nc.sync.dma_start(
                            y_out[si * P:(si + 1) * P, csl], yt[:, csl])
                    if nck == 1:
                        del yts[(sc, s8)]
                return go

            def emit_pv(sc, h, kcp, et):
              with nc.named_scope("attn"):
                jc, po = h // 2, (h % 2) * DH
                if kcp == 0:
                    pus[(sc, h)] = psum_u.tile([P, 512], F32, tag="u",
                                               name=f"pu_{sc}_{h}")
                pu = pus[(sc, h)]
                for dk in range(2):
                    kc = kcp * 2 + dk
                    nc.tensor.matmul(
                        pu[:],
                        mm(vpp[:, kc, h * P:(h + 1) * P]),
                        mm(et[:, dk * 512:(dk + 1) * 512]),
                        start=(kcp == 0 and dk == 0),
                        stop=(kcp == n_kcp - 1 and dk == 1))
                if kcp == n_kcp - 1:
                    cT = cts[sc]
                    rt = small.tile([DH, 512], F32, tag="rt",
                                    name=f"rt_{sc}_{h}")
                    nc.vector.reciprocal(rt[:], pu[DH:P, :])
                    nc.vector.tensor_mul(
                        r(cT[po:po + DH, jc, :]), pu[0:DH, :], rt[:])
                    del pus[(sc, h)]
                    if h == HPC - 1:
                        cT_done = cts.pop(sc)
                        for s8 in range(4):
                            for nck in range(2):
                                fill.append(
                                    oproj_piece(sc, cT_done, s8, nck))

            pend = []
            import collections as _c
            fill = _c.deque()

            def emit_attn_unit(sc, h, kcp):
              with nc.named_scope("attn"):
                if (h, kcp) == (0, 0):
                    cts[sc] = ct_pool.tile([P, j_chunks, 512], F32,
                                           tag="cT", name=f"cT_{sc}")
                jc, po = h // 2, (h % 2) * DH
                ps = psum_s.tile([P, 1024], F32, tag="s",
                                 name=f"ps_{sc}_{h}_{kcp}")
                for dk in range(2):
                    kc = kcp * 2 + dk
                    nc.tensor.matmul(
                        ps[:, dk * 512:(dk + 1) * 512],
                        mm(kT[po:po + DH, jc, kc * P:(kc + 1) * P]),
                        mm(qT[po:po + DH, jc, sc * 512:(sc + 1) * 512]),
                        start=True, stop=True)
                et = epool.tile([P, 1024], F32, tag="e",
                                name=f"et_{sc}_{h}_{kcp}")
                nc.scalar.activation(r(et[:]), ps[:], EXP, scale=0.125)
                pend.append((sc, h, kcp, et))
                if len(pend) > lag:
                    emit_pv(*pend.pop(0))

            # -- attention: 4 chunks of 512 queries. The next chunk's
            #    Q-projection and the previous chunk's out-projection are
            #    drip-fed from the fill queue, one ~850ns piece per unit,
            #    so the PE stays busy while ACT works through the exps --
            # slab3's K and V projections are drip-fed at the start of
            # attention (kT slab3 is first read at unit 6, vpp rows 12-15
            # at unit 6+lag), so the attention stream starts ~5us earlier
            kp3 = jproj_pieces(wk_sb, x2Ts[3], 3, kT, bk_sb,
                               "kproj", step=4)
            vp3 = [vproj_piece(3, q, pool=psum_mm, tag="mm")
                   for q in range(4)]
            for a, b in zip(kp3, vp3):
                fill.append(a)
                fill.append(b)
            for sc in range(n_slabs):
                if sc + 1 < n_slabs:
                    fill.extend(qproj_pieces(sc + 1))
                for h in range(HPC):
                    for kcp in range(n_kcp):
                        emit_attn_unit(sc, h, kcp)
                        u = h * n_kcp + kcp
                        if sc == 0 and u < 8 and u % 2 == 0:
                            # double-pop: slab3's deferred K/V projections
                            # must land before units 6..10 consume them
                            for _ in range(min(2, len(fill))):
                                fill.popleft()()
                        elif fill and (
                                u % 2 == 0 if sc < n_slabs - 1
                                else h == HPC - 1):
                            fill.popleft()()
            with nc.named_scope("attn"):
                for args in pend:
                    emit_pv(*args)
                    if fill:
                        fill.popleft()()
                while fill:
                    fill.popleft()()

            # -- sum partials across the 4 cores of this batch --
            # Two half-sized ReduceScatters: the first depends only on the
            # first 1024 rows, so it overlaps the second half's attention.
            if with_collective:
                half = seq // 2                 # 1024 rows per collective
                qr = seq // GROUPS // 2         # 256 rows per rank per half
                for ci in range(2):
                    ysc = dram.tile([qr, D], F32, tag="yout",
                                    name=f"ysc_{ci}")
                    nc.gpsimd.collective_compute(
                        "ReduceScatter",
                        mybir.AluOpType.add,
                        replica_groups=[[0, 1, 2, 3], [4, 5, 6, 7]],
                        ins=[ybounce[ci * half:(ci + 1) * half, :].opt()],
                        outs=[ysc[:].opt()],
                    )
                    nc.sync.dma_start(y_out[ci * qr:(ci + 1) * qr, :], ysc[:])
            # (no-collective build: y_out rows were written directly by
            # emit_oproj's sc==0 DMAs)

    nc.compile()
    return nc


def _get_program(seq=SEQ):
    if seq not in _cached:
        _cached[seq] = _build_program(seq)
    return _cached[seq]


def make_in_maps(x1, x2, Wq, bq, Wk, bk, Wv, bv, Wo, bo):
    """Per-core input dicts for the SPMD program (x and Wqkv host-cast to
    bf16; attention itself stays f32r on-chip)."""
    import ml_dtypes
    bf16 = ml_dtypes.bfloat16
    x1 = np.asarray(x1, np.float32).astype(bf16)
    x2 = np.asarray(x2, np.float32).astype(bf16)
    Wqh = np.asarray(Wq, np.float32).astype(bf16)
    Wkh = np.asarray(Wk, np.float32).astype(bf16)
    Wvh = np.asarray(Wv, np.float32).astype(bf16)
    Wo = np.asarray(Wo, np.float32)
    bq = np.asarray(bq, np.float32)
    bk = np.asarray(bk, np.float32)
    in_maps = []
    for c in range(N_CORES):
        b, g = c // GROUPS, c % GROUPS
        js = slice(g * JG, (g + 1) * JG)
        in_maps.append({
            "x1r": np.ascontiguousarray(x1[b]),
            "x2r": np.ascontiguousarray(x2[b]),
            "wq": np.ascontiguousarray(Wqh[:, js]),
            "wk": np.ascontiguousarray(Wkh[:, js]),
            "wv": np.ascontiguousarray(Wvh[:, js]),
            "wo": np.ascontiguousarray(Wo[js, :]),
            "bqr": np.ascontiguousarray(bq[js].reshape(2, P).T),
            "bkr": np.ascontiguousarray(bk[js].reshape(2, P).T),
        })
    return in_maps


def assemble(results, Wv_bias_fix):
    """results: list of per-core {'y_out': [seq//GROUPS, D]}.

    y_out rows [0:q) = rank's quarter of input rows [0:seq/2);
    rows [q:2q) = rank's quarter of input rows [seq/2:seq)."""
    seq = results[0]["y_out"].shape[0] * GROUPS
    q = seq // GROUPS // 2
    Y = np.empty((B, seq, D), np.float32)
    for c in range(N_CORES):
        b, rr = c // GROUPS, c % GROUPS
        yo = results[c]["y_out"]
        Y[b, rr * q:(rr + 1) * q, :] = yo[:q]
        Y[b, seq // 2 + rr * q:seq // 2 + (rr + 1) * q, :] = yo[q:]
    Y += Wv_bias_fix
    return Y


def kernel(x1, x2, Wq, bq, Wk, bk, Wv, bv, Wo, bo):
    from concourse.bass_utils import run_bass_kernel_spmd

    Wo = np.asarray(Wo, np.float32)
    bv = np.asarray(bv, np.float32)
    bo = np.asarray(bo, np.float32)

    nc = _get_program(SEQ)
    in_maps = make_in_maps(x1, x2, Wq, bq, Wk, bk, Wv, bv, Wo, bo)
    res = run_bass_kernel_spmd(nc, in_maps, core_ids=list(range(N_CORES)))
    fix = (bv @ Wo + bo).astype(np.float32)
    return assemble(res.results, fix)
